# revision 32
# baseline (speedup 1.0000x reference)
"""Multi-head latent attention (MLA) Bass kernel for 8 TRN2 NeuronCores.

Sharding: tensor-parallel over heads x data-parallel over batch.
Core c (0..7) owns batch b = c//4 and head group g = c%4 (8 heads of 32).

No collectives: a NEFF containing any collective_compute runs the PE at
~263ns/512col instead of ~216ns (a global ~22% clock penalty), so each
core computes the FULL latent for its batch itself (+83us of matmul)
instead of sharding latent + AllGather. The latent stays resident in
SBUF between phase A and B (no DRAM round trip).

Phases (single core):
  A: per seq chunk j: latent chains (Wc) + q-proj chains (Wq) share the
     same hsT chunk tiles; q RoPE'd on DVE into qT [Dh, S] per head.
  B: kT (RoPE'd, [Dh, S]) + v from SBUF-resident latent; RoPE split
     across Scalar (kb copy), DVE (psum-reading muls) and GpSimd
     (bf16 mul + add) so no single engine paces PE; Wo prefetched.
  C: per query chunk: scores -> exp (ACT) -> DVE tree-sum denominator +
     ones-matmul partition reduction; attention fused with
     out-projection; out-proj chains of the previous chunk are
     interleaved INTO the score loop (4 mms per score slot) so PE never
     waits on the exp drain.
Host sums the 4 partials per batch.

Compute dtype: bf16 on the TensorE inputs, fp32 PSUM accumulation.
"""

import sys

for _p in ("/opt/trn_rl_repo", "/root/.axon_site/_ro/trn_rl_repo"):
    if _p not in sys.path:
        sys.path.insert(0, _p)

import numpy as np
import ml_dtypes

import concourse.bacc as bacc
import concourse.mybir as mybir
import concourse.tile as tile
from concourse.bass_utils import run_bass_kernel_spmd

BF = mybir.dt.bfloat16
F32 = mybir.dt.float32
BF_NP = ml_dtypes.bfloat16

# Full-problem constants (hardcoded per the self-contained-kernel contract).
D_MODEL = 4096
D_LATENT = 512
NUM_HEADS = 32
HEAD_DIM = 128
ROPE_THETA = 10000.0
BATCH, SEQ = 2, 2048
N_CORES = 8
HEADS_PER_CORE = NUM_HEADS // 4  # 4 head groups x 2 batches = 8 cores


def build_nc(S=SEQ, D=D_MODEL, L=D_LATENT, H=HEADS_PER_CORE, Dh=HEAD_DIM,
             NA=512, NC=512):
    """Build the single-core Bass program (SPMD across 8 cores)."""
    assert S % NA == 0 and S % 128 == 0 and D % 128 == 0 and L % 128 == 0
    KD = D // 128     # contraction chunks over d_model
    LD = L // 128     # contraction chunks over d_latent
    JA = S // NA      # seq chunks in projection phase
    JC = S // NC      # seq chunks in attention phase
    SK = S // 128     # key-position chunks
    HD1 = H * Dh      # this core's total head width (1024)
    ND = D // NC      # output-column chunks

    nc = bacc.Bacc("TRN2", target_bir_lowering=False)

    hsT_d = nc.declare_dram_parameter("hsT", [D, S], BF, isOutput=False)
    wq_d = nc.declare_dram_parameter("Wq", [D, HD1], BF, isOutput=False)
    wc_d = nc.declare_dram_parameter("Wc", [D, L], BF, isOutput=False)
    wk_d = nc.declare_dram_parameter("Wk", [L, HD1], BF, isOutput=False)
    wv_d = nc.declare_dram_parameter("Wv", [L, HD1], BF, isOutput=False)
    wo_d = nc.declare_dram_parameter("Wo", [HD1, D], BF, isOutput=False)
    cosq_d = nc.declare_dram_parameter("cosq", [Dh, S], BF, isOutput=False)
    sinq_d = nc.declare_dram_parameter("sinq", [Dh, S], BF, isOutput=False)
    cosk_d = nc.declare_dram_parameter("cosk", [Dh, S], BF, isOutput=False)
    sink_d = nc.declare_dram_parameter("sink", [Dh, S], BF, isOutput=False)
    out_d = nc.declare_dram_parameter("out", [S, D], BF, isOutput=True)

    Exp = mybir.ActivationFunctionType.Exp
    half = Dh // 2

    with tile.TileContext(nc) as tc:
        with tc.tile_pool(name="consts", bufs=1) as const_pool:
            ones_sk = const_pool.tile([128, 128], BF)
            nc.vector.memset(ones_sk[:], 1.0)
            with tc.tile_pool(name="qT", bufs=1) as qT_pool, \
                 tc.tile_pool(name="lat", bufs=1) as lat_pool:
                qT_t = [qT_pool.tile([Dh, S], BF, name=f"qT{h}") for h in range(H)]
                # latent stays SBUF-resident from A through B (outer pool,
                # fresh addresses: B's kT chains have no WAR wait on A's end)
                lat_t = [lat_pool.tile([128, S], BF, name=f"lat{ld}")
                         for ld in range(LD)]

                # == Phase A: full latent + qT (with RoPE), shared hs tiles ==
                with tc.tile_pool(name="wqA", bufs=1) as wqA_pool, \
                     tc.tile_pool(name="wcA", bufs=1) as wcA_pool, \
                     tc.tile_pool(name="hsA", bufs=KD + 8) as hsA_pool, \
                     tc.tile_pool(name="ropeq", bufs=1) as ropeq_pool, \
                     tc.tile_pool(name="tmpA", bufs=2) as tmpA_pool, \
                     tc.tile_pool(name="psA", bufs=8, space="PSUM") as psA_pool:

                    HW2 = HD1 // 2
                    wq_t = [[wqA_pool.tile([128, HW2], BF,
                                           name=f"wq{p}_{kd}")
                             for kd in range(KD)] for p in range(2)]
                    wc_t = [wcA_pool.tile([128, L], BF, name=f"wc{kd}")
                            for kd in range(KD)]
                    cosq_sb = ropeq_pool.tile([Dh, S], BF)
                    sinq_sb = ropeq_pool.tile([Dh, S], BF)

                    def emit_rope_q(j, h, ps):
                        jj = slice(j * NA, (j + 1) * NA)
                        t1 = tmpA_pool.tile([128, NA], BF, tag="t1",
                                            name=f"t1q{j}_{h}")
                        t2 = tmpA_pool.tile([128, NA], BF, tag="t2",
                                            name=f"t2q{j}_{h}")
                        nc.vector.tensor_mul(t1[:], ps[:], cosq_sb[:, jj])
                        nc.vector.tensor_mul(t2[0:half, :], ps[half:Dh, :],
                                             sinq_sb[0:half, jj])
                        nc.vector.tensor_mul(t2[half:Dh, :], ps[0:half, :],
                                             sinq_sb[half:Dh, jj])
                        nc.vector.tensor_add(qT_t[h][:, jj], t1[:], t2[:])

                    def emit_lsb(j, ps_l):
                        # latent straight into the resident SBUF tiles
                        jj = slice(j * NA, (j + 1) * NA)
                        for ld in range(LD):
                            nc.scalar.copy(lat_t[ld][:, jj], ps_l[ld][:])

                    # ---- j0: cold start is DMA-paced, so interleave the
                    # latent chains with q chains h0-3 kd-outer: per kd PE
                    # does 8 matmuls (1.7us) while the rings deliver the
                    # (wc, hs, wq_lo) triple for kd+1.
                    j0 = slice(0, NA)
                    hs_c0 = []
                    for kd in range(KD):
                        nc.sync.dma_start(
                            out=wc_t[kd][:],
                            in_=wc_d[kd * 128:(kd + 1) * 128, :])
                        t = hsA_pool.tile([128, NA], BF, tag="hsA",
                                          name=f"hsA_0_{kd}")
                        nc.scalar.dma_start(
                            out=t[:], in_=hsT_d[kd * 128:(kd + 1) * 128, j0])
                        hs_c0.append(t)
                        (nc.sync if kd % 2 == 0 else nc.scalar).dma_start(
                            out=wq_t[0][kd][:],
                            in_=wq_d[kd * 128:(kd + 1) * 128, 0:HW2])
                    nc.gpsimd.dma_start(out=cosq_sb[:], in_=cosq_d[:])
                    nc.gpsimd.dma_start(out=sinq_sb[:], in_=sinq_d[:])
                    for kd in range(KD):
                        nc.gpsimd.dma_start(
                            out=wq_t[1][kd][:],
                            in_=wq_d[kd * 128:(kd + 1) * 128, HW2:HD1])
                    ps_l0 = [psA_pool.tile([128, NA], F32, tag="psA",
                                           name=f"psL0_{ld}")
                             for ld in range(LD)]
                    ps_q0 = [psA_pool.tile([128, NA], F32, tag="psA",
                                           name=f"psQ0_{h}")
                             for h in range(4)]
                    for kd in range(KD):
                        for ld in range(LD):
                            nc.tensor.matmul(
                                ps_l0[ld][:],
                                wc_t[kd][:, ld * 128:(ld + 1) * 128],
                                hs_c0[kd][:],
                                start=(kd == 0), stop=(kd == KD - 1))
                        for h in range(4):
                            nc.tensor.matmul(
                                ps_q0[h][:],
                                wq_t[0][kd][:, h * Dh:h * Dh + Dh],
                                hs_c0[kd][:],
                                start=(kd == 0), stop=(kd == KD - 1))
                    emit_lsb(0, ps_l0)
                    for h in range(4):
                        emit_rope_q(0, h, ps_q0[h])
                    for h in range(4, H):
                        hp, hq = divmod(h * Dh, HW2)
                        ps = psA_pool.tile([128, NA], F32, tag="psA",
                                           name=f"psQ0_{h}")
                        for kd in range(KD):
                            nc.tensor.matmul(
                                ps[:], wq_t[hp][kd][:, hq:hq + Dh],
                                hs_c0[kd][:],
                                start=(kd == 0), stop=(kd == KD - 1))
                        emit_rope_q(0, h, ps)

                    # ---- j1..j3: steady state; hs chunks for j+1 prefetch
                    # freely during j (hsA ring holds two full j's).
                    for j in range(1, JA):
                        jj = slice(j * NA, (j + 1) * NA)
                        hs_ch = []
                        for kd in range(KD):
                            t = hsA_pool.tile([128, NA], BF, tag="hsA",
                                              name=f"hsA_{j}_{kd}")
                            (nc.sync if kd % 2 == 0 else nc.scalar).dma_start(
                                out=t[:], in_=hsT_d[kd * 128:(kd + 1) * 128, jj])
                            hs_ch.append(t)

                        # latent chains for this j (kd-outer, 4 chains)
                        ps_l = [psA_pool.tile([128, NA], F32, tag="psA",
                                              name=f"psL{j}_{ld}")
                                for ld in range(LD)]
                        for kd in range(KD):
                            for ld in range(LD):
                                nc.tensor.matmul(
                                    ps_l[ld][:],
                                    wc_t[kd][:, ld * 128:(ld + 1) * 128],
                                    hs_ch[kd][:],
                                    start=(kd == 0), stop=(kd == KD - 1))
                        emit_lsb(j, ps_l)

                        # q-proj chains + RoPE
                        for h in range(H):
                            hp, hq = divmod(h * Dh, HW2)
                            ps = psA_pool.tile([128, NA], F32, tag="psA",
                                               name=f"psQ{j}_{h}")
                            for kd in range(KD):
                                nc.tensor.matmul(
                                    ps[:], wq_t[hp][kd][:, hq:hq + Dh],
                                    hs_ch[kd][:],
                                    start=(kd == 0), stop=(kd == KD - 1))
                            emit_rope_q(j, h, ps)

                # ========== Phase B: kT (with RoPE) + v; prefetch Wo ========
                with tc.tile_pool(name="kT", bufs=1) as kT_pool, \
                     tc.tile_pool(name="v", bufs=1) as v_pool, \
                     tc.tile_pool(name="wo", bufs=1) as wo_pool:
                    kT_t = [kT_pool.tile([Dh, S], BF, name=f"kT{h}")
                            for h in range(H)]
                    v_t = [v_pool.tile([128, HD1], BF, name=f"v{i}")
                           for i in range(SK)]
                    wo_t = [wo_pool.tile([128, D], BF, name=f"wo{h}")
                            for h in range(H)]

                    # One PSUM pool spans B and C' so C's score banks are
                    # disjoint from B's working banks (no WAR delay):
                    # tags: pb 2x1 bank (B kT/v psums + C' out-proj),
                    #       sc 2x2 banks (scores), pv 2x1 bank (pv + denom).
                    ps_cm = tc.tile_pool(name="pswork", bufs=2, space="PSUM")
                    ps_pool = ps_cm.__enter__()
                    with tc.tile_pool(name="wkv", bufs=1) as wkv_pool, \
                         tc.tile_pool(name="ropek", bufs=1) as ropek_pool, \
                         tc.tile_pool(name="tmpB", bufs=1) as tmpB_pool:

                        wk_t = [wkv_pool.tile([128, HD1], BF, name=f"wk{ld}")
                                for ld in range(LD)]
                        wv_t = [wkv_pool.tile([128, HD1], BF, name=f"wv{ld}")
                                for ld in range(LD)]
                        lq_t = lat_t
                        # wk on the gpsimd ring: idle at A's tail, so wk is
                        # the only DMA gating the first kT chain.
                        for ld in range(LD):
                            nc.gpsimd.dma_start(
                                out=wk_t[ld][:],
                                in_=wk_d[ld * 128:(ld + 1) * 128, :])
                        cosk_sb = ropek_pool.tile([Dh, S], BF)
                        sink_sb = ropek_pool.tile([Dh, S], BF)
                        nc.scalar.dma_start(out=cosk_sb[:], in_=cosk_d[:])
                        nc.scalar.dma_start(out=sink_sb[:], in_=sink_d[:])
                        for ld in range(LD):
                            nc.sync.dma_start(
                                out=wv_t[ld][:],
                                in_=wv_d[ld * 128:(ld + 1) * 128, :])
                        # Wo prefetch early: B has ~60us, Wo is 8MB over two
                        # queues; first consumer is C' jc=1 (~70us later).
                        for hh in range(H):
                            eng = nc.sync if hh % 2 == 0 else nc.scalar
                            eng.dma_start(
                                out=wo_t[hh][:],
                                in_=wo_d[hh * 128:(hh + 1) * 128, :])

                        # kT: h outer so each head's kT completes early and
                        # unblocks that head's score matmuls in C'. Blocks
                        # are 1024 wide on the idle "sc" psum banks to halve
                        # per-op overheads. RoPE work is split: kb copy on
                        # Scalar, psum-reading muls + t1 on DVE, final add
                        # on GpSimd -- no single engine paces PE. v-expand
                        # chains (from SBUF-resident lq) interleave to keep
                        # PE dense.
                        NB = 1024
                        for h in range(H):
                            for j in range(S // NB):
                                jj = slice(j * NB, (j + 1) * NB)
                                ps = ps_pool.tile([128, NB], F32, tag="sc",
                                                  name=f"psK{h}_{j}")
                                for p in range(2):
                                    pp = slice(p * 512, (p + 1) * 512)
                                    jp = slice(j * NB + p * 512,
                                               j * NB + (p + 1) * 512)
                                    for ld in range(LD):
                                        nc.tensor.matmul(
                                            ps[:, pp],
                                            wk_t[ld][:, h * Dh:(h + 1) * Dh],
                                            lq_t[ld][:, jp],
                                            start=(ld == 0),
                                            stop=(ld == LD - 1))
                                t1 = tmpB_pool.tile([128, NB], BF, tag="t1b",
                                                    name=f"t1k{h}_{j}")
                                t2 = tmpB_pool.tile([128, NB], BF, tag="t2b",
                                                    name=f"t2k{h}_{j}")
                                nc.vector.tensor_mul(t2[0:half, :],
                                                     ps[half:Dh, :],
                                                     sink_sb[0:half, jj])
                                nc.vector.tensor_mul(t2[half:Dh, :],
                                                     ps[0:half, :],
                                                     sink_sb[half:Dh, jj])
                                nc.vector.tensor_mul(t1[:], ps[:],
                                                     cosk_sb[:, jj])
                                nc.gpsimd.tensor_add(kT_t[h][:, jj],
                                                     t1[:], t2[:])
                                # one v seq-tile after each double-block
                                i = 2 * h + j
                                io = slice(i * 128, (i + 1) * 128)
                                for cch in range(HD1 // 512):
                                    cc = slice(cch * 512, (cch + 1) * 512)
                                    ps = ps_pool.tile([128, 512], F32,
                                                      tag="pb",
                                                      name=f"psV{i}_{cch}")
                                    for ld in range(LD):
                                        nc.tensor.matmul(
                                            ps[:], lq_t[ld][:, io],
                                            wv_t[ld][:, cc],
                                            start=(ld == 0),
                                            stop=(ld == LD - 1))
                                    nc.scalar.copy(v_t[i][:, cc], ps[:])

                    # ===== Phase C': attention fused with out-projection =====
                    with tc.tile_pool(name="ET", bufs=4) as et_pool, \
                         tc.tile_pool(name="esum", bufs=1) as esum_pool, \
                         tc.tile_pool(name="rinv", bufs=1) as rinv_pool, \
                         tc.tile_pool(name="ats", bufs=2 * H) as ats_pool, \
                         tc.tile_pool(name="outst", bufs=1) as outst_pool:

                        assert SK % 2 == 0

                        def emit_norm(jc, h, esf, pv, ats_t):
                            # denominator matmul + reciprocal + normalize.
                            # Deferred one head so the matmul never waits on
                            # the DVE esum chain.
                            rr = ps_pool.tile([128, NC], F32, tag="pv",
                                              name=f"rr{h}_{jc}")
                            nc.tensor.matmul(rr[:], ones_sk[:], esf[:],
                                             start=True, stop=True)
                            rbs = rinv_pool.tile([128, NC], F32, tag="rbs",
                                                 name=f"rbs{h}_{jc}")
                            nc.vector.reciprocal_approx_fast(rbs[:], rr[:])
                            ats = ats_pool.tile([Dh, NC], BF, tag="ats",
                                                name=f"ats{h}_{jc}")
                            nc.vector.tensor_mul(ats[:], pv[:], rbs[:])
                            ats_t.append(ats)

                        # out-projection chains are emitted in 4-matmul
                        # groups via this generator-like cursor so they can
                        # be interleaved into the score loop.
                        class DChain:
                            def __init__(self):
                                self.jobs = []  # (djc, tl, ncol)
                                self.pos = 0
                                self.ps = None
                                self.ats = None

                            def add(self, djc, tl, ncol, d_ats):
                                self.jobs.append((djc, tl, ncol, d_ats))

                            def emit(self, n):
                                # emit n matmuls worth of chain work
                                while n > 0 and (self.jobs or self.ps):
                                    if self.ps is None:
                                        djc, tl, ncol, d_ats = self.jobs.pop(0)
                                        self.cur = (djc, tl, ncol)
                                        self.ats = d_ats
                                        self.pos = 0
                                        self.ps = ps_pool.tile(
                                            [128, NC], F32, tag="pb",
                                            name=f"psD{djc}_{tl}_{ncol}")
                                    djc, tl, ncol = self.cur
                                    toff = slice(tl * 128, (tl + 1) * 128)
                                    cc = slice(ncol * NC, (ncol + 1) * NC)
                                    take = min(n, H - self.pos)
                                    for h in range(self.pos, self.pos + take):
                                        nc.tensor.matmul(
                                            self.ps[:], self.ats[h][:, toff],
                                            wo_t[h][:, cc],
                                            start=(h == 0), stop=(h == H - 1))
                                    self.pos += take
                                    n -= take
                                    if self.pos == H:
                                        tt = slice(djc * NC + tl * 128,
                                                   djc * NC + (tl + 1) * 128)
                                        st = outst_pool.tile(
                                            [128, NC], BF, tag="outst",
                                            name=f"outst{djc}_{tl}_{ncol}")
                                        nc.scalar.copy(st[:], self.ps[:])
                                        nc.sync.dma_start(out=out_d[tt, cc],
                                                          in_=st[:])
                                        self.ps = None

                        dchain = DChain()
                        prev_ats = None
                        for jc in range(JC):
                            jj = slice(jc * NC, (jc + 1) * NC)
                            ats_t = []
                            pending = None
                            for h in range(H):
                                if prev_ats is not None:
                                    for k in range(4):
                                        idx = h * 4 + k
                                        dchain.add(jc - 1, idx // ND,
                                                   idx % ND, prev_ats)
                                ets = []
                                for i2 in range(SK // 2):
                                    ps2 = ps_pool.tile(
                                        [128, 2 * NC], F32, tag="sc",
                                        name=f"sc{h}_{jc}_{i2}")
                                    for p in range(2):
                                        i = i2 * 2 + p
                                        nc.tensor.matmul(
                                            ps2[:, p * NC:(p + 1) * NC],
                                            kT_t[h][:, i * 128:(i + 1) * 128],
                                            qT_t[h][:, jj],
                                            start=True, stop=True)
                                    # 4 out-proj matmuls of the PREVIOUS
                                    # query chunk between score pairs: ready
                                    # PE work that absorbs the exp drain.
                                    dchain.emit(4)
                                    et = et_pool.tile([128, 2 * NC], BF,
                                                      tag="ET",
                                                      name=f"et{h}_{jc}_{i2}")
                                    nc.scalar.activation(et[:], ps2[:], Exp)
                                    ets.append(et)
                                # previous head's norm ops go FIRST so its
                                # ats never queues behind this head's esum
                                # chain on DVE (pv buffer reuse waits on ats)
                                if pending is not None:
                                    emit_norm(jc, h - 1, *pending, ats_t)
                                # DVE tree-sum of the exp tiles for the
                                # softmax denominator.
                                es = esum_pool.tile([128, 2 * NC], BF,
                                                    tag="es", name=f"es{h}_{jc}")
                                nc.vector.tensor_add(es[:], ets[0][:], ets[1][:])
                                for i2 in range(2, SK // 2):
                                    nc.vector.tensor_add(es[:], es[:], ets[i2][:])
                                esf = esum_pool.tile([128, NC], BF, tag="esf",
                                                     name=f"esf{h}_{jc}")
                                nc.vector.tensor_add(esf[:], es[:, 0:NC],
                                                     es[:, NC:2 * NC])
                                pv = ps_pool.tile([Dh, NC], F32, tag="pv",
                                                    name=f"pv{h}_{jc}")
                                for i2 in range(SK // 2):
                                    for p in range(2):
                                        i = i2 * 2 + p
                                        sl = ets[i2][:, p * NC:(p + 1) * NC]
                                        nc.tensor.matmul(
                                            pv[:],
                                            v_t[i][:, h * Dh:(h + 1) * Dh],
                                            sl, start=(i == 0),
                                            stop=(i == SK - 1))
                                pending = (esf, pv)
                            emit_norm(jc, H - 1, *pending, ats_t)
                            prev_ats = ats_t

                        # out-projection for the last query chunk
                        for tl in range(NC // 128):
                            for ncol in range(ND):
                                dchain.add(JC - 1, tl, ncol, prev_ats)
                        dchain.emit(10 ** 9)
                    ps_cm.__exit__(None, None, None)

    nc.compile()
    return nc


def host_inputs(hidden_states, Wq, Wc, Wk, Wv, Wo, S=SEQ, Dh=HEAD_DIM,
                heads_per_core=HEADS_PER_CORE, n_cores=N_CORES):
    """Shard + preprocess full fp32 inputs into per-core bf16 in_maps."""
    scale = 1.0 / np.sqrt(Dh)
    pos = np.arange(S, dtype=np.float32)
    inv_freq = 1.0 / (ROPE_THETA ** (np.arange(0, Dh, 2, dtype=np.float32) / Dh))
    freqs = pos[:, None] * inv_freq
    emb = np.concatenate([freqs, freqs], axis=-1)      # [S, Dh]
    cosT = np.cos(emb).T.copy()                        # [Dh, S]
    sinT = np.sin(emb).T.copy()
    sinT[: Dh // 2] *= -1.0                            # sign baked for the swap trick
    cosq = (cosT * scale).astype(BF_NP)
    sinq = (sinT * scale).astype(BF_NP)
    cosk = cosT.astype(BF_NP)
    sink = sinT.astype(BF_NP)

    hw = heads_per_core * Dh
    in_maps = []
    for c in range(n_cores):
        b, g = divmod(c, 4)
        cols = slice(g * hw, (g + 1) * hw)
        in_maps.append({
            "hsT": np.ascontiguousarray(hidden_states[b].T).astype(BF_NP),
            "Wq": np.ascontiguousarray(Wq[:, cols]).astype(BF_NP),
            "Wc": Wc.astype(BF_NP),
            "Wk": np.ascontiguousarray(Wk[:, cols]).astype(BF_NP),
            "Wv": np.ascontiguousarray(Wv[:, cols]).astype(BF_NP),
            "Wo": np.ascontiguousarray(Wo[cols, :]).astype(BF_NP),
            "cosq": cosq, "sinq": sinq, "cosk": cosk, "sink": sink,
        })
    return in_maps


_NC_CACHE = {}


def kernel(hidden_states, Wq, Wc, Wk, Wv, Wo):
    hidden_states = np.asarray(hidden_states, dtype=np.float32)
    if "nc" not in _NC_CACHE:
        _NC_CACHE["nc"] = build_nc()
    nc = _NC_CACHE["nc"]
    in_maps = host_inputs(hidden_states, np.asarray(Wq, np.float32),
                          np.asarray(Wc, np.float32), np.asarray(Wk, np.float32),
                          np.asarray(Wv, np.float32), np.asarray(Wo, np.float32))
    res = run_bass_kernel_spmd(nc, in_maps, list(range(N_CORES))).results
    B, S, D = BATCH, SEQ, D_MODEL
    out = np.zeros((B, S, D), dtype=np.float32)
    for c in range(N_CORES):
        out[c // 4] += res[c]["out"]
    return out


# revision 33
# speedup vs baseline: 1.0660x; 1.0660x over previous
"""Multi-head latent attention (MLA) Bass kernel for 8 TRN2 NeuronCores.

Sharding: tensor-parallel over heads x data-parallel over batch.
Core c (0..7) owns batch b = c//4 and head group g = c%4 (8 heads of 32).

No collectives: a NEFF containing any collective_compute runs the PE at
~263ns/512col instead of ~216ns (a global ~22% clock penalty), so each
core computes the FULL latent for its batch itself (+83us of matmul)
instead of sharding latent + AllGather. The latent stays resident in
SBUF between phase A and B (no DRAM round trip).

Phases (single core):
  A: per seq chunk j: latent chains (Wc) + q-proj chains (Wq) share the
     same hsT chunk tiles; q RoPE'd on DVE into qT [Dh, S] per head.
  B: kT (RoPE'd, [Dh, S]) + v from SBUF-resident latent; RoPE split
     across Scalar (kb copy), DVE (psum-reading muls) and GpSimd
     (bf16 mul + add) so no single engine paces PE; Wo prefetched.
  C: per query chunk: scores -> exp (ACT) -> DVE tree-sum denominator +
     ones-matmul partition reduction; attention fused with
     out-projection; out-proj chains of the previous chunk are
     interleaved INTO the score loop (4 mms per score slot) so PE never
     waits on the exp drain.
Host sums the 4 partials per batch.

Compute dtype: bf16 on the TensorE inputs, fp32 PSUM accumulation.
"""

import sys

for _p in ("/opt/trn_rl_repo", "/root/.axon_site/_ro/trn_rl_repo"):
    if _p not in sys.path:
        sys.path.insert(0, _p)

import numpy as np
import ml_dtypes

import concourse.bacc as bacc
import concourse.mybir as mybir
import concourse.tile as tile
from concourse.bass_utils import run_bass_kernel_spmd

BF = mybir.dt.bfloat16
F32 = mybir.dt.float32
BF_NP = ml_dtypes.bfloat16

# Full-problem constants (hardcoded per the self-contained-kernel contract).
D_MODEL = 4096
D_LATENT = 512
NUM_HEADS = 32
HEAD_DIM = 128
ROPE_THETA = 10000.0
BATCH, SEQ = 2, 2048
N_CORES = 8
HEADS_PER_CORE = NUM_HEADS // 4  # 4 head groups x 2 batches = 8 cores


def build_nc(S=SEQ, D=D_MODEL, L=D_LATENT, H=HEADS_PER_CORE, Dh=HEAD_DIM,
             NA=512, NC=512):
    """Build the single-core Bass program (SPMD across 8 cores)."""
    assert S % NA == 0 and S % 128 == 0 and D % 128 == 0 and L % 128 == 0
    KD = D // 128     # contraction chunks over d_model
    LD = L // 128     # contraction chunks over d_latent
    JA = S // NA      # seq chunks in projection phase
    JC = S // NC      # seq chunks in attention phase
    SK = S // 128     # key-position chunks
    HD1 = H * Dh      # this core's total head width (1024)
    ND = D // NC      # output-column chunks

    nc = bacc.Bacc("TRN2", target_bir_lowering=False)

    hsT_d = nc.declare_dram_parameter("hsT", [D, S], BF, isOutput=False)
    wq_d = nc.declare_dram_parameter("Wq", [D, HD1], BF, isOutput=False)
    wc_d = nc.declare_dram_parameter("Wc", [D, L], BF, isOutput=False)
    wk_d = nc.declare_dram_parameter("Wk", [L, HD1], BF, isOutput=False)
    wv_d = nc.declare_dram_parameter("Wv", [L, HD1], BF, isOutput=False)
    wo_d = nc.declare_dram_parameter("Wo", [HD1, D], BF, isOutput=False)
    cosq_d = nc.declare_dram_parameter("cosq", [Dh, S], BF, isOutput=False)
    sinq_d = nc.declare_dram_parameter("sinq", [Dh, S], BF, isOutput=False)
    cosk_d = nc.declare_dram_parameter("cosk", [Dh, S], BF, isOutput=False)
    sink_d = nc.declare_dram_parameter("sink", [Dh, S], BF, isOutput=False)
    out_d = nc.declare_dram_parameter("out", [S, D], BF, isOutput=True)
    latq_d = nc.dram_tensor("latq_dram", [L, S], BF)

    Exp = mybir.ActivationFunctionType.Exp
    half = Dh // 2

    with tile.TileContext(nc) as tc:
        with tc.tile_pool(name="consts", bufs=1) as const_pool:
            ones_sk = const_pool.tile([128, 128], BF)
            nc.vector.memset(ones_sk[:], 1.0)
            with tc.tile_pool(name="qT", bufs=1) as qT_pool:
                qT_t = [qT_pool.tile([Dh, S], BF, name=f"qT{h}") for h in range(H)]

                # == Phase A: full latent + qT (with RoPE), shared hs tiles ==
                with tc.tile_pool(name="wqA", bufs=1) as wqA_pool, \
                     tc.tile_pool(name="wcA", bufs=1) as wcA_pool, \
                     tc.tile_pool(name="hsA", bufs=KD + 8) as hsA_pool, \
                     tc.tile_pool(name="ropeq", bufs=1) as ropeq_pool, \
                     tc.tile_pool(name="tmpA", bufs=2) as tmpA_pool, \
                     tc.tile_pool(name="psA", bufs=8, space="PSUM") as psA_pool:

                    HW2 = HD1 // 2
                    wq_t = [[wqA_pool.tile([128, HW2], BF,
                                           name=f"wq{p}_{kd}")
                             for kd in range(KD)] for p in range(2)]
                    wc_t = [wcA_pool.tile([128, L], BF, name=f"wc{kd}")
                            for kd in range(KD)]
                    cosq_sb = ropeq_pool.tile([Dh, S], BF)
                    sinq_sb = ropeq_pool.tile([Dh, S], BF)

                    def emit_rope_q(j, h, ps):
                        jj = slice(j * NA, (j + 1) * NA)
                        t1 = tmpA_pool.tile([128, NA], BF, tag="t1",
                                            name=f"t1q{j}_{h}")
                        t2 = tmpA_pool.tile([128, NA], BF, tag="t2",
                                            name=f"t2q{j}_{h}")
                        nc.vector.tensor_mul(t1[:], ps[:], cosq_sb[:, jj])
                        nc.vector.tensor_mul(t2[0:half, :], ps[half:Dh, :],
                                             sinq_sb[0:half, jj])
                        nc.vector.tensor_mul(t2[half:Dh, :], ps[0:half, :],
                                             sinq_sb[half:Dh, jj])
                        nc.vector.tensor_add(qT_t[h][:, jj], t1[:], t2[:])

                    def emit_lsb(j, ps_l):
                        # latent -> DRAM (freed from SBUF before C'; phase B
                        # reloads it into a B-scoped pool)
                        jj = slice(j * NA, (j + 1) * NA)
                        for ld in range(LD):
                            lsb = tmpA_pool.tile([128, NA], BF, tag="lsb",
                                                 name=f"lsb{j}_{ld}")
                            nc.scalar.copy(lsb[:], ps_l[ld][:])
                            nc.sync.dma_start(
                                out=latq_d[ld * 128:(ld + 1) * 128, jj],
                                in_=lsb[:])

                    # ---- j0: cold start is DMA-paced, so interleave the
                    # latent chains with q chains h0-3 kd-outer: per kd PE
                    # does 8 matmuls (1.7us) while the rings deliver the
                    # (wc, hs, wq_lo) triple for kd+1.
                    j0 = slice(0, NA)
                    hs_c0 = []
                    for kd in range(KD):
                        nc.sync.dma_start(
                            out=wc_t[kd][:],
                            in_=wc_d[kd * 128:(kd + 1) * 128, :])
                        t = hsA_pool.tile([128, NA], BF, tag="hsA",
                                          name=f"hsA_0_{kd}")
                        nc.scalar.dma_start(
                            out=t[:], in_=hsT_d[kd * 128:(kd + 1) * 128, j0])
                        hs_c0.append(t)
                        (nc.sync if kd % 2 == 0 else nc.scalar).dma_start(
                            out=wq_t[0][kd][:],
                            in_=wq_d[kd * 128:(kd + 1) * 128, 0:HW2])
                    nc.gpsimd.dma_start(out=cosq_sb[:], in_=cosq_d[:])
                    nc.gpsimd.dma_start(out=sinq_sb[:], in_=sinq_d[:])
                    for kd in range(KD):
                        nc.gpsimd.dma_start(
                            out=wq_t[1][kd][:],
                            in_=wq_d[kd * 128:(kd + 1) * 128, HW2:HD1])
                    ps_l0 = [psA_pool.tile([128, NA], F32, tag="psA",
                                           name=f"psL0_{ld}")
                             for ld in range(LD)]
                    ps_q0 = [psA_pool.tile([128, NA], F32, tag="psA",
                                           name=f"psQ0_{h}")
                             for h in range(4)]
                    for kd in range(KD):
                        for ld in range(LD):
                            nc.tensor.matmul(
                                ps_l0[ld][:],
                                wc_t[kd][:, ld * 128:(ld + 1) * 128],
                                hs_c0[kd][:],
                                start=(kd == 0), stop=(kd == KD - 1))
                        for h in range(4):
                            nc.tensor.matmul(
                                ps_q0[h][:],
                                wq_t[0][kd][:, h * Dh:h * Dh + Dh],
                                hs_c0[kd][:],
                                start=(kd == 0), stop=(kd == KD - 1))
                    emit_lsb(0, ps_l0)
                    for h in range(4):
                        emit_rope_q(0, h, ps_q0[h])
                    for h in range(4, H):
                        hp, hq = divmod(h * Dh, HW2)
                        ps = psA_pool.tile([128, NA], F32, tag="psA",
                                           name=f"psQ0_{h}")
                        for kd in range(KD):
                            nc.tensor.matmul(
                                ps[:], wq_t[hp][kd][:, hq:hq + Dh],
                                hs_c0[kd][:],
                                start=(kd == 0), stop=(kd == KD - 1))
                        emit_rope_q(0, h, ps)

                    # ---- j1..j3: steady state; hs chunks for j+1 prefetch
                    # freely during j (hsA ring holds two full j's).
                    for j in range(1, JA):
                        jj = slice(j * NA, (j + 1) * NA)
                        hs_ch = []
                        for kd in range(KD):
                            t = hsA_pool.tile([128, NA], BF, tag="hsA",
                                              name=f"hsA_{j}_{kd}")
                            (nc.sync if kd % 2 == 0 else nc.scalar).dma_start(
                                out=t[:], in_=hsT_d[kd * 128:(kd + 1) * 128, jj])
                            hs_ch.append(t)

                        # latent chains for this j (kd-outer, 4 chains)
                        ps_l = [psA_pool.tile([128, NA], F32, tag="psA",
                                              name=f"psL{j}_{ld}")
                                for ld in range(LD)]
                        for kd in range(KD):
                            for ld in range(LD):
                                nc.tensor.matmul(
                                    ps_l[ld][:],
                                    wc_t[kd][:, ld * 128:(ld + 1) * 128],
                                    hs_ch[kd][:],
                                    start=(kd == 0), stop=(kd == KD - 1))
                        emit_lsb(j, ps_l)

                        # q-proj chains + RoPE
                        for h in range(H):
                            hp, hq = divmod(h * Dh, HW2)
                            ps = psA_pool.tile([128, NA], F32, tag="psA",
                                               name=f"psQ{j}_{h}")
                            for kd in range(KD):
                                nc.tensor.matmul(
                                    ps[:], wq_t[hp][kd][:, hq:hq + Dh],
                                    hs_ch[kd][:],
                                    start=(kd == 0), stop=(kd == KD - 1))
                            emit_rope_q(j, h, ps)

                # ========== Phase B: kT (with RoPE) + v; prefetch Wo ========
                with tc.tile_pool(name="kT", bufs=1) as kT_pool, \
                     tc.tile_pool(name="v", bufs=1) as v_pool, \
                     tc.tile_pool(name="wo", bufs=1) as wo_pool:
                    kT_t = [kT_pool.tile([Dh, S], BF, name=f"kT{h}")
                            for h in range(H)]
                    v_t = [v_pool.tile([128, HD1], BF, name=f"v{i}")
                           for i in range(SK)]
                    wo_t = [wo_pool.tile([128, D], BF, name=f"wo{h}")
                            for h in range(H)]

                    # One PSUM pool spans B and C' so C's score banks are
                    # disjoint from B's working banks (no WAR delay):
                    # tags: pb 2x1 bank (B kT/v psums + C' out-proj),
                    #       sc 2x2 banks (scores), pv 2x1 bank (pv + denom).
                    ps_cm = tc.tile_pool(name="pswork", bufs=2, space="PSUM")
                    ps_pool = ps_cm.__enter__()
                    with tc.tile_pool(name="latB", bufs=1) as latB_pool, \
                         tc.tile_pool(name="wkv", bufs=1) as wkv_pool, \
                         tc.tile_pool(name="ropek", bufs=1) as ropek_pool, \
                         tc.tile_pool(name="tmpB", bufs=1) as tmpB_pool:

                        lq_t = [latB_pool.tile([128, S], BF, name=f"latB{ld}")
                                for ld in range(LD)]
                        wk_t = [wkv_pool.tile([128, HD1], BF, name=f"wk{ld}")
                                for ld in range(LD)]
                        wv_t = [wkv_pool.tile([128, HD1], BF, name=f"wv{ld}")
                                for ld in range(LD)]
                        # latB + wk lead every ring: their target addresses
                        # free early (over wq_t[0], done when heads 0-3 of
                        # the last A chunk finish), so the reload overlaps
                        # A's tail instead of head-of-line blocking behind
                        # late-WAR entries like cosk/wo.
                        rings = [nc.sync, nc.scalar, nc.gpsimd]
                        for jh in range(2):
                            js = slice(jh * 1024, (jh + 1) * 1024)
                            for ld in range(LD):
                                rings[(jh * LD + ld) % 3].dma_start(
                                    out=lq_t[ld][:, js],
                                    in_=latq_d[ld * 128:(ld + 1) * 128, js])
                        for ld in range(LD):
                            rings[ld % 3].dma_start(
                                out=wk_t[ld][:],
                                in_=wk_d[ld * 128:(ld + 1) * 128, :])
                        cosk_sb = ropek_pool.tile([Dh, S], BF)
                        sink_sb = ropek_pool.tile([Dh, S], BF)
                        nc.scalar.dma_start(out=cosk_sb[:], in_=cosk_d[:])
                        nc.scalar.dma_start(out=sink_sb[:], in_=sink_d[:])
                        for ld in range(LD):
                            nc.sync.dma_start(
                                out=wv_t[ld][:],
                                in_=wv_d[ld * 128:(ld + 1) * 128, :])
                        # Wo prefetch early: B has ~60us, Wo is 8MB over two
                        # queues; first consumer is C' jc=1 (~70us later).
                        for hh in range(H):
                            eng = nc.sync if hh % 2 == 0 else nc.scalar
                            eng.dma_start(
                                out=wo_t[hh][:],
                                in_=wo_d[hh * 128:(hh + 1) * 128, :])

                        # kT: h outer so each head's kT completes early and
                        # unblocks that head's score matmuls in C'. Blocks
                        # are 1024 wide on the idle "sc" psum banks to halve
                        # per-op overheads. RoPE work is split: kb copy on
                        # Scalar, psum-reading muls + t1 on DVE, final add
                        # on GpSimd -- no single engine paces PE. v-expand
                        # chains (from SBUF-resident lq) interleave to keep
                        # PE dense.
                        NB = 1024
                        for h in range(H):
                            for j in range(S // NB):
                                jj = slice(j * NB, (j + 1) * NB)
                                ps = ps_pool.tile([128, NB], F32, tag="sc",
                                                  name=f"psK{h}_{j}")
                                for p in range(2):
                                    pp = slice(p * 512, (p + 1) * 512)
                                    jp = slice(j * NB + p * 512,
                                               j * NB + (p + 1) * 512)
                                    for ld in range(LD):
                                        nc.tensor.matmul(
                                            ps[:, pp],
                                            wk_t[ld][:, h * Dh:(h + 1) * Dh],
                                            lq_t[ld][:, jp],
                                            start=(ld == 0),
                                            stop=(ld == LD - 1))
                                kb = tmpB_pool.tile([128, NB], BF, tag="kb",
                                                    name=f"kb{h}_{j}")
                                nc.scalar.copy(kb[:], ps[:])
                                t1 = tmpB_pool.tile([128, NB], BF, tag="t1b",
                                                    name=f"t1k{h}_{j}")
                                t2 = tmpB_pool.tile([128, NB], BF, tag="t2b",
                                                    name=f"t2k{h}_{j}")
                                nc.vector.tensor_mul(t2[0:half, :],
                                                     ps[half:Dh, :],
                                                     sink_sb[0:half, jj])
                                nc.vector.tensor_mul(t2[half:Dh, :],
                                                     ps[0:half, :],
                                                     sink_sb[half:Dh, jj])
                                nc.vector.tensor_mul(t1[:], kb[:],
                                                     cosk_sb[:, jj])
                                nc.gpsimd.tensor_add(kT_t[h][:, jj],
                                                     t1[:], t2[:])
                                # one v seq-tile after each double-block
                                i = 2 * h + j
                                io = slice(i * 128, (i + 1) * 128)
                                for cch in range(HD1 // 512):
                                    cc = slice(cch * 512, (cch + 1) * 512)
                                    ps = ps_pool.tile([128, 512], F32,
                                                      tag="pb",
                                                      name=f"psV{i}_{cch}")
                                    for ld in range(LD):
                                        nc.tensor.matmul(
                                            ps[:], lq_t[ld][:, io],
                                            wv_t[ld][:, cc],
                                            start=(ld == 0),
                                            stop=(ld == LD - 1))
                                    nc.scalar.copy(v_t[i][:, cc], ps[:])

                    # ===== Phase C': attention fused with out-projection =====
                    with tc.tile_pool(name="ET", bufs=8) as et_pool, \
                         tc.tile_pool(name="esum", bufs=1) as esum_pool, \
                         tc.tile_pool(name="rinv", bufs=1) as rinv_pool, \
                         tc.tile_pool(name="ats", bufs=2 * H) as ats_pool, \
                         tc.tile_pool(name="outst", bufs=2) as outst_pool:

                        assert SK % 2 == 0

                        def emit_norm(jc, h, esf, pv, ats_t):
                            # denominator matmul + reciprocal + normalize.
                            # Deferred one head so the matmul never waits on
                            # the DVE esum chain.
                            rr = ps_pool.tile([128, NC], F32, tag="pv",
                                              name=f"rr{h}_{jc}")
                            nc.tensor.matmul(rr[:], ones_sk[:], esf[:],
                                             start=True, stop=True)
                            rbs = rinv_pool.tile([128, NC], F32, tag="rbs",
                                                 name=f"rbs{h}_{jc}")
                            nc.vector.reciprocal_approx_fast(rbs[:], rr[:])
                            ats = ats_pool.tile([Dh, NC], BF, tag="ats",
                                                name=f"ats{h}_{jc}")
                            nc.vector.tensor_mul(ats[:], pv[:], rbs[:])
                            ats_t.append(ats)

                        # out-projection chains are emitted in 4-matmul
                        # groups via this generator-like cursor so they can
                        # be interleaved into the score loop.
                        class DChain:
                            def __init__(self):
                                self.jobs = []  # (djc, tl, ncol)
                                self.pos = 0
                                self.ps = None
                                self.ats = None

                            def add(self, djc, tl, ncol, d_ats):
                                self.jobs.append((djc, tl, ncol, d_ats))

                            def emit(self, n):
                                # emit n matmuls worth of chain work
                                while n > 0 and (self.jobs or self.ps):
                                    if self.ps is None:
                                        djc, tl, ncol, d_ats = self.jobs.pop(0)
                                        self.cur = (djc, tl, ncol)
                                        self.ats = d_ats
                                        self.pos = 0
                                        self.ps = ps_pool.tile(
                                            [128, NC], F32, tag="pb",
                                            name=f"psD{djc}_{tl}_{ncol}")
                                    djc, tl, ncol = self.cur
                                    toff = slice(tl * 128, (tl + 1) * 128)
                                    cc = slice(ncol * NC, (ncol + 1) * NC)
                                    take = min(n, H - self.pos)
                                    for h in range(self.pos, self.pos + take):
                                        nc.tensor.matmul(
                                            self.ps[:], self.ats[h][:, toff],
                                            wo_t[h][:, cc],
                                            start=(h == 0), stop=(h == H - 1))
                                    self.pos += take
                                    n -= take
                                    if self.pos == H:
                                        tt = slice(djc * NC + tl * 128,
                                                   djc * NC + (tl + 1) * 128)
                                        st = outst_pool.tile(
                                            [128, NC], BF, tag="outst",
                                            name=f"outst{djc}_{tl}_{ncol}")
                                        nc.scalar.copy(st[:], self.ps[:])
                                        nc.sync.dma_start(out=out_d[tt, cc],
                                                          in_=st[:])
                                        self.ps = None

                        dchain = DChain()
                        prev_ats = None
                        for jc in range(JC):
                            jj = slice(jc * NC, (jc + 1) * NC)
                            ats_t = []
                            pending = None
                            for h in range(H):
                                if prev_ats is not None:
                                    for k in range(4):
                                        idx = h * 4 + k
                                        dchain.add(jc - 1, idx // ND,
                                                   idx % ND, prev_ats)
                                ets = []
                                for i2 in range(SK // 2):
                                    ps2 = ps_pool.tile(
                                        [128, 2 * NC], F32, tag="sc",
                                        name=f"sc{h}_{jc}_{i2}")
                                    for p in range(2):
                                        i = i2 * 2 + p
                                        nc.tensor.matmul(
                                            ps2[:, p * NC:(p + 1) * NC],
                                            kT_t[h][:, i * 128:(i + 1) * 128],
                                            qT_t[h][:, jj],
                                            start=True, stop=True)
                                    # 4 out-proj matmuls of the PREVIOUS
                                    # query chunk between score pairs: ready
                                    # PE work that absorbs the exp drain.
                                    dchain.emit(4)
                                    et = et_pool.tile([128, 2 * NC], BF,
                                                      tag="ET",
                                                      name=f"et{h}_{jc}_{i2}")
                                    nc.scalar.activation(et[:], ps2[:], Exp)
                                    ets.append(et)
                                # previous head's norm ops go FIRST so its
                                # ats never queues behind this head's esum
                                # chain on DVE (pv buffer reuse waits on ats)
                                if pending is not None:
                                    emit_norm(jc, h - 1, *pending, ats_t)
                                # DVE tree-sum of the exp tiles for the
                                # softmax denominator.
                                es = esum_pool.tile([128, 2 * NC], BF,
                                                    tag="es", name=f"es{h}_{jc}")
                                nc.vector.tensor_add(es[:], ets[0][:], ets[1][:])
                                for i2 in range(2, SK // 2):
                                    nc.vector.tensor_add(es[:], es[:], ets[i2][:])
                                esf = esum_pool.tile([128, NC], BF, tag="esf",
                                                     name=f"esf{h}_{jc}")
                                nc.vector.tensor_add(esf[:], es[:, 0:NC],
                                                     es[:, NC:2 * NC])
                                pv = ps_pool.tile([Dh, NC], F32, tag="pv",
                                                    name=f"pv{h}_{jc}")
                                for i2 in range(SK // 2):
                                    for p in range(2):
                                        i = i2 * 2 + p
                                        sl = ets[i2][:, p * NC:(p + 1) * NC]
                                        nc.tensor.matmul(
                                            pv[:],
                                            v_t[i][:, h * Dh:(h + 1) * Dh],
                                            sl, start=(i == 0),
                                            stop=(i == SK - 1))
                                pending = (esf, pv)
                            emit_norm(jc, H - 1, *pending, ats_t)
                            prev_ats = ats_t

                        # out-projection for the last query chunk
                        for tl in range(NC // 128):
                            for ncol in range(ND):
                                dchain.add(JC - 1, tl, ncol, prev_ats)
                        dchain.emit(10 ** 9)
                    ps_cm.__exit__(None, None, None)

    nc.compile()
    return nc


def host_inputs(hidden_states, Wq, Wc, Wk, Wv, Wo, S=SEQ, Dh=HEAD_DIM,
                heads_per_core=HEADS_PER_CORE, n_cores=N_CORES):
    """Shard + preprocess full fp32 inputs into per-core bf16 in_maps."""
    scale = 1.0 / np.sqrt(Dh)
    pos = np.arange(S, dtype=np.float32)
    inv_freq = 1.0 / (ROPE_THETA ** (np.arange(0, Dh, 2, dtype=np.float32) / Dh))
    freqs = pos[:, None] * inv_freq
    emb = np.concatenate([freqs, freqs], axis=-1)      # [S, Dh]
    cosT = np.cos(emb).T.copy()                        # [Dh, S]
    sinT = np.sin(emb).T.copy()
    sinT[: Dh // 2] *= -1.0                            # sign baked for the swap trick
    cosq = (cosT * scale).astype(BF_NP)
    sinq = (sinT * scale).astype(BF_NP)
    cosk = cosT.astype(BF_NP)
    sink = sinT.astype(BF_NP)

    hw = heads_per_core * Dh
    in_maps = []
    for c in range(n_cores):
        b, g = divmod(c, 4)
        cols = slice(g * hw, (g + 1) * hw)
        in_maps.append({
            "hsT": np.ascontiguousarray(hidden_states[b].T).astype(BF_NP),
            "Wq": np.ascontiguousarray(Wq[:, cols]).astype(BF_NP),
            "Wc": Wc.astype(BF_NP),
            "Wk": np.ascontiguousarray(Wk[:, cols]).astype(BF_NP),
            "Wv": np.ascontiguousarray(Wv[:, cols]).astype(BF_NP),
            "Wo": np.ascontiguousarray(Wo[cols, :]).astype(BF_NP),
            "cosq": cosq, "sinq": sinq, "cosk": cosk, "sink": sink,
        })
    return in_maps


_NC_CACHE = {}


def kernel(hidden_states, Wq, Wc, Wk, Wv, Wo):
    hidden_states = np.asarray(hidden_states, dtype=np.float32)
    if "nc" not in _NC_CACHE:
        _NC_CACHE["nc"] = build_nc()
    nc = _NC_CACHE["nc"]
    in_maps = host_inputs(hidden_states, np.asarray(Wq, np.float32),
                          np.asarray(Wc, np.float32), np.asarray(Wk, np.float32),
                          np.asarray(Wv, np.float32), np.asarray(Wo, np.float32))
    res = run_bass_kernel_spmd(nc, in_maps, list(range(N_CORES))).results
    B, S, D = BATCH, SEQ, D_MODEL
    out = np.zeros((B, S, D), dtype=np.float32)
    for c in range(N_CORES):
        out[c // 4] += res[c]["out"]
    return out


# revision 34
# speedup vs baseline: 1.0694x; 1.0032x over previous
"""Multi-head latent attention (MLA) Bass kernel for 8 TRN2 NeuronCores.

Sharding: tensor-parallel over heads x data-parallel over batch.
Core c (0..7) owns batch b = c//4 and head group g = c%4 (8 heads of 32).

No collectives: a NEFF containing any collective_compute runs the PE at
~263ns/512col instead of ~216ns (a global ~22% clock penalty), so each
core computes the FULL latent for its batch itself (+83us of matmul)
instead of sharding latent + AllGather. The latent stays resident in
SBUF between phase A and B (no DRAM round trip).

Phases (single core):
  A: per seq chunk j: latent chains (Wc) + q-proj chains (Wq) share the
     same hsT chunk tiles; q RoPE'd on DVE into qT [Dh, S] per head.
  B: kT (RoPE'd, [Dh, S]) + v from SBUF-resident latent; RoPE split
     across Scalar (kb copy), DVE (psum-reading muls) and GpSimd
     (bf16 mul + add) so no single engine paces PE; Wo prefetched.
  C: per query chunk: scores -> exp (ACT) -> DVE tree-sum denominator +
     ones-matmul partition reduction; attention fused with
     out-projection; out-proj chains of the previous chunk are
     interleaved INTO the score loop (4 mms per score slot) so PE never
     waits on the exp drain.
Host sums the 4 partials per batch.

Compute dtype: bf16 on the TensorE inputs, fp32 PSUM accumulation.
"""

import sys

for _p in ("/opt/trn_rl_repo", "/root/.axon_site/_ro/trn_rl_repo"):
    if _p not in sys.path:
        sys.path.insert(0, _p)

import numpy as np
import ml_dtypes

import concourse.bacc as bacc
import concourse.mybir as mybir
import concourse.tile as tile
from concourse.bass_utils import run_bass_kernel_spmd

BF = mybir.dt.bfloat16
F32 = mybir.dt.float32
BF_NP = ml_dtypes.bfloat16

# Full-problem constants (hardcoded per the self-contained-kernel contract).
D_MODEL = 4096
D_LATENT = 512
NUM_HEADS = 32
HEAD_DIM = 128
ROPE_THETA = 10000.0
BATCH, SEQ = 2, 2048
N_CORES = 8
HEADS_PER_CORE = NUM_HEADS // 4  # 4 head groups x 2 batches = 8 cores


def build_nc(S=SEQ, D=D_MODEL, L=D_LATENT, H=HEADS_PER_CORE, Dh=HEAD_DIM,
             NA=512, NC=512):
    """Build the single-core Bass program (SPMD across 8 cores)."""
    assert S % NA == 0 and S % 128 == 0 and D % 128 == 0 and L % 128 == 0
    KD = D // 128     # contraction chunks over d_model
    LD = L // 128     # contraction chunks over d_latent
    JA = S // NA      # seq chunks in projection phase
    JC = S // NC      # seq chunks in attention phase
    SK = S // 128     # key-position chunks
    HD1 = H * Dh      # this core's total head width (1024)
    ND = D // NC      # output-column chunks

    nc = bacc.Bacc("TRN2", target_bir_lowering=False)

    hsT_d = nc.declare_dram_parameter("hsT", [D, S], BF, isOutput=False)
    wq_d = nc.declare_dram_parameter("Wq", [D, HD1], BF, isOutput=False)
    wc_d = nc.declare_dram_parameter("Wc", [D, L], BF, isOutput=False)
    wk_d = nc.declare_dram_parameter("Wk", [L, HD1], BF, isOutput=False)
    wv_d = nc.declare_dram_parameter("Wv", [L, HD1], BF, isOutput=False)
    wo_d = nc.declare_dram_parameter("Wo", [HD1, D], BF, isOutput=False)
    cosq_d = nc.declare_dram_parameter("cosq", [Dh, S], BF, isOutput=False)
    sinq_d = nc.declare_dram_parameter("sinq", [Dh, S], BF, isOutput=False)
    cosk_d = nc.declare_dram_parameter("cosk", [Dh, S], BF, isOutput=False)
    sink_d = nc.declare_dram_parameter("sink", [Dh, S], BF, isOutput=False)
    out_d = nc.declare_dram_parameter("out", [S, D], BF, isOutput=True)
    latq_d = nc.dram_tensor("latq_dram", [L, S], BF)

    Exp = mybir.ActivationFunctionType.Exp
    half = Dh // 2

    with tile.TileContext(nc) as tc:
        with tc.tile_pool(name="consts", bufs=1) as const_pool:
            ones_sk = const_pool.tile([128, 128], BF)
            nc.vector.memset(ones_sk[:], 1.0)
            with tc.tile_pool(name="qT", bufs=1) as qT_pool:
                qT_t = [qT_pool.tile([Dh, S], BF, name=f"qT{h}") for h in range(H)]

                # == Phase A: full latent + qT (with RoPE), shared hs tiles ==
                with tc.tile_pool(name="wcA", bufs=1) as wcA_pool, \
                     tc.tile_pool(name="wqA", bufs=1) as wqA_pool, \
                     tc.tile_pool(name="hsA", bufs=KD + 8) as hsA_pool, \
                     tc.tile_pool(name="ropeq", bufs=1) as ropeq_pool, \
                     tc.tile_pool(name="tmpA", bufs=2) as tmpA_pool, \
                     tc.tile_pool(name="psA", bufs=8, space="PSUM") as psA_pool:

                    HW2 = HD1 // 2
                    wq_t = [[wqA_pool.tile([128, HW2], BF,
                                           name=f"wq{p}_{kd}")
                             for kd in range(KD)] for p in range(2)]
                    wc_t = [wcA_pool.tile([128, L], BF, name=f"wc{kd}")
                            for kd in range(KD)]
                    cosq_sb = ropeq_pool.tile([Dh, S], BF)
                    sinq_sb = ropeq_pool.tile([Dh, S], BF)

                    def emit_rope_q(j, h, ps):
                        jj = slice(j * NA, (j + 1) * NA)
                        t1 = tmpA_pool.tile([128, NA], BF, tag="t1",
                                            name=f"t1q{j}_{h}")
                        t2 = tmpA_pool.tile([128, NA], BF, tag="t2",
                                            name=f"t2q{j}_{h}")
                        nc.vector.tensor_mul(t1[:], ps[:], cosq_sb[:, jj])
                        nc.vector.tensor_mul(t2[0:half, :], ps[half:Dh, :],
                                             sinq_sb[0:half, jj])
                        nc.vector.tensor_mul(t2[half:Dh, :], ps[0:half, :],
                                             sinq_sb[half:Dh, jj])
                        nc.vector.tensor_add(qT_t[h][:, jj], t1[:], t2[:])

                    def emit_lsb(j, ps_l):
                        # latent -> DRAM (freed from SBUF before C'; phase B
                        # reloads it into a B-scoped pool)
                        jj = slice(j * NA, (j + 1) * NA)
                        for ld in range(LD):
                            lsb = tmpA_pool.tile([128, NA], BF, tag="lsb",
                                                 name=f"lsb{j}_{ld}")
                            nc.scalar.copy(lsb[:], ps_l[ld][:])
                            nc.sync.dma_start(
                                out=latq_d[ld * 128:(ld + 1) * 128, jj],
                                in_=lsb[:])

                    # ---- j0: cold start is DMA-paced, so interleave the
                    # latent chains with q chains h0-3 kd-outer: per kd PE
                    # does 8 matmuls (1.7us) while the rings deliver the
                    # (wc, hs, wq_lo) triple for kd+1.
                    j0 = slice(0, NA)
                    hs_c0 = []
                    for kd in range(KD):
                        nc.sync.dma_start(
                            out=wc_t[kd][:],
                            in_=wc_d[kd * 128:(kd + 1) * 128, :])
                        t = hsA_pool.tile([128, NA], BF, tag="hsA",
                                          name=f"hsA_0_{kd}")
                        nc.scalar.dma_start(
                            out=t[:], in_=hsT_d[kd * 128:(kd + 1) * 128, j0])
                        hs_c0.append(t)
                        (nc.sync if kd % 2 == 0 else nc.scalar).dma_start(
                            out=wq_t[0][kd][:],
                            in_=wq_d[kd * 128:(kd + 1) * 128, 0:HW2])
                    nc.gpsimd.dma_start(out=cosq_sb[:], in_=cosq_d[:])
                    nc.gpsimd.dma_start(out=sinq_sb[:], in_=sinq_d[:])
                    for kd in range(KD):
                        nc.gpsimd.dma_start(
                            out=wq_t[1][kd][:],
                            in_=wq_d[kd * 128:(kd + 1) * 128, HW2:HD1])
                    ps_l0 = [psA_pool.tile([128, NA], F32, tag="psA",
                                           name=f"psL0_{ld}")
                             for ld in range(LD)]
                    ps_q0 = [psA_pool.tile([128, NA], F32, tag="psA",
                                           name=f"psQ0_{h}")
                             for h in range(4)]
                    for kd in range(KD):
                        for ld in range(LD):
                            nc.tensor.matmul(
                                ps_l0[ld][:],
                                wc_t[kd][:, ld * 128:(ld + 1) * 128],
                                hs_c0[kd][:],
                                start=(kd == 0), stop=(kd == KD - 1))
                        for h in range(4):
                            nc.tensor.matmul(
                                ps_q0[h][:],
                                wq_t[0][kd][:, h * Dh:h * Dh + Dh],
                                hs_c0[kd][:],
                                start=(kd == 0), stop=(kd == KD - 1))
                    emit_lsb(0, ps_l0)
                    for h in range(4):
                        emit_rope_q(0, h, ps_q0[h])
                    for h in range(4, H):
                        hp, hq = divmod(h * Dh, HW2)
                        ps = psA_pool.tile([128, NA], F32, tag="psA",
                                           name=f"psQ0_{h}")
                        for kd in range(KD):
                            nc.tensor.matmul(
                                ps[:], wq_t[hp][kd][:, hq:hq + Dh],
                                hs_c0[kd][:],
                                start=(kd == 0), stop=(kd == KD - 1))
                        emit_rope_q(0, h, ps)

                    # ---- j1..j3: steady state; hs chunks for j+1 prefetch
                    # freely during j (hsA ring holds two full j's).
                    for j in range(1, JA):
                        jj = slice(j * NA, (j + 1) * NA)
                        hs_ch = []
                        for kd in range(KD):
                            t = hsA_pool.tile([128, NA], BF, tag="hsA",
                                              name=f"hsA_{j}_{kd}")
                            (nc.sync if kd % 2 == 0 else nc.scalar).dma_start(
                                out=t[:], in_=hsT_d[kd * 128:(kd + 1) * 128, jj])
                            hs_ch.append(t)

                        # latent chains for this j (kd-outer, 4 chains)
                        ps_l = [psA_pool.tile([128, NA], F32, tag="psA",
                                              name=f"psL{j}_{ld}")
                                for ld in range(LD)]
                        for kd in range(KD):
                            for ld in range(LD):
                                nc.tensor.matmul(
                                    ps_l[ld][:],
                                    wc_t[kd][:, ld * 128:(ld + 1) * 128],
                                    hs_ch[kd][:],
                                    start=(kd == 0), stop=(kd == KD - 1))
                        emit_lsb(j, ps_l)

                        # q-proj chains + RoPE
                        for h in range(H):
                            hp, hq = divmod(h * Dh, HW2)
                            ps = psA_pool.tile([128, NA], F32, tag="psA",
                                               name=f"psQ{j}_{h}")
                            for kd in range(KD):
                                nc.tensor.matmul(
                                    ps[:], wq_t[hp][kd][:, hq:hq + Dh],
                                    hs_ch[kd][:],
                                    start=(kd == 0), stop=(kd == KD - 1))
                            emit_rope_q(j, h, ps)

                # ========== Phase B: kT (with RoPE) + v; prefetch Wo ========
                with tc.tile_pool(name="kT", bufs=1) as kT_pool, \
                     tc.tile_pool(name="v", bufs=1) as v_pool, \
                     tc.tile_pool(name="wo", bufs=1) as wo_pool:
                    kT_t = [kT_pool.tile([Dh, S], BF, name=f"kT{h}")
                            for h in range(H)]
                    v_t = [v_pool.tile([128, HD1], BF, name=f"v{i}")
                           for i in range(SK)]
                    wo_t = [wo_pool.tile([128, D], BF, name=f"wo{h}")
                            for h in range(H)]

                    # One PSUM pool spans B and C' so C's score banks are
                    # disjoint from B's working banks (no WAR delay):
                    # tags: pb 2x1 bank (B kT/v psums + C' out-proj),
                    #       sc 2x2 banks (scores), pv 2x1 bank (pv + denom).
                    ps_cm = tc.tile_pool(name="pswork", bufs=2, space="PSUM")
                    ps_pool = ps_cm.__enter__()
                    with tc.tile_pool(name="latB", bufs=1) as latB_pool, \
                         tc.tile_pool(name="wkv", bufs=1) as wkv_pool, \
                         tc.tile_pool(name="ropek", bufs=1) as ropek_pool, \
                         tc.tile_pool(name="tmpB", bufs=1) as tmpB_pool:

                        lq_t = [latB_pool.tile([128, S], BF, name=f"latB{ld}")
                                for ld in range(LD)]
                        wk_t = [wkv_pool.tile([128, HD1], BF, name=f"wk{ld}")
                                for ld in range(LD)]
                        wv_t = [wkv_pool.tile([128, HD1], BF, name=f"wv{ld}")
                                for ld in range(LD)]
                        # latB + wk lead every ring: their target addresses
                        # free early (over wq_t[0], done when heads 0-3 of
                        # the last A chunk finish), so the reload overlaps
                        # A's tail instead of head-of-line blocking behind
                        # late-WAR entries like cosk/wo.
                        rings = [nc.sync, nc.scalar, nc.gpsimd]
                        for jh in range(2):
                            js = slice(jh * 1024, (jh + 1) * 1024)
                            for ld in range(LD):
                                rings[(jh * LD + ld) % 3].dma_start(
                                    out=lq_t[ld][:, js],
                                    in_=latq_d[ld * 128:(ld + 1) * 128, js])
                        for ld in range(LD):
                            rings[ld % 3].dma_start(
                                out=wk_t[ld][:],
                                in_=wk_d[ld * 128:(ld + 1) * 128, :])
                        cosk_sb = ropek_pool.tile([Dh, S], BF)
                        sink_sb = ropek_pool.tile([Dh, S], BF)
                        nc.scalar.dma_start(out=cosk_sb[:], in_=cosk_d[:])
                        nc.scalar.dma_start(out=sink_sb[:], in_=sink_d[:])
                        for ld in range(LD):
                            nc.sync.dma_start(
                                out=wv_t[ld][:],
                                in_=wv_d[ld * 128:(ld + 1) * 128, :])
                        # Wo prefetch early: B has ~60us, Wo is 8MB over two
                        # queues; first consumer is C' jc=1 (~70us later).
                        for hh in range(H):
                            eng = nc.sync if hh % 2 == 0 else nc.scalar
                            eng.dma_start(
                                out=wo_t[hh][:],
                                in_=wo_d[hh * 128:(hh + 1) * 128, :])

                        # kT: h outer so each head's kT completes early and
                        # unblocks that head's score matmuls in C'. Blocks
                        # are 1024 wide on the idle "sc" psum banks to halve
                        # per-op overheads. RoPE work is split: kb copy on
                        # Scalar, psum-reading muls + t1 on DVE, final add
                        # on GpSimd -- no single engine paces PE. v-expand
                        # chains (from SBUF-resident lq) interleave to keep
                        # PE dense.
                        NB = 1024
                        for h in range(H):
                            for j in range(S // NB):
                                jj = slice(j * NB, (j + 1) * NB)
                                ps = ps_pool.tile([128, NB], F32, tag="sc",
                                                  name=f"psK{h}_{j}")
                                for p in range(2):
                                    pp = slice(p * 512, (p + 1) * 512)
                                    jp = slice(j * NB + p * 512,
                                               j * NB + (p + 1) * 512)
                                    for ld in range(LD):
                                        nc.tensor.matmul(
                                            ps[:, pp],
                                            wk_t[ld][:, h * Dh:(h + 1) * Dh],
                                            lq_t[ld][:, jp],
                                            start=(ld == 0),
                                            stop=(ld == LD - 1))
                                kb = tmpB_pool.tile([128, NB], BF, tag="kb",
                                                    name=f"kb{h}_{j}")
                                nc.scalar.copy(kb[:], ps[:])
                                t1 = tmpB_pool.tile([128, NB], BF, tag="t1b",
                                                    name=f"t1k{h}_{j}")
                                t2 = tmpB_pool.tile([128, NB], BF, tag="t2b",
                                                    name=f"t2k{h}_{j}")
                                nc.vector.tensor_mul(t2[0:half, :],
                                                     ps[half:Dh, :],
                                                     sink_sb[0:half, jj])
                                nc.vector.tensor_mul(t2[half:Dh, :],
                                                     ps[0:half, :],
                                                     sink_sb[half:Dh, jj])
                                nc.vector.tensor_mul(t1[:], kb[:],
                                                     cosk_sb[:, jj])
                                nc.gpsimd.tensor_add(kT_t[h][:, jj],
                                                     t1[:], t2[:])
                                # one v seq-tile after each double-block
                                i = 2 * h + j
                                io = slice(i * 128, (i + 1) * 128)
                                for cch in range(HD1 // 512):
                                    cc = slice(cch * 512, (cch + 1) * 512)
                                    ps = ps_pool.tile([128, 512], F32,
                                                      tag="pb",
                                                      name=f"psV{i}_{cch}")
                                    for ld in range(LD):
                                        nc.tensor.matmul(
                                            ps[:], lq_t[ld][:, io],
                                            wv_t[ld][:, cc],
                                            start=(ld == 0),
                                            stop=(ld == LD - 1))
                                    nc.scalar.copy(v_t[i][:, cc], ps[:])

                    # ===== Phase C': attention fused with out-projection =====
                    with tc.tile_pool(name="ET", bufs=8) as et_pool, \
                         tc.tile_pool(name="esum", bufs=1) as esum_pool, \
                         tc.tile_pool(name="rinv", bufs=1) as rinv_pool, \
                         tc.tile_pool(name="ats", bufs=2 * H) as ats_pool, \
                         tc.tile_pool(name="outst", bufs=2) as outst_pool:

                        assert SK % 2 == 0

                        def emit_norm(jc, h, esf, pv, ats_t):
                            # denominator matmul + reciprocal + normalize.
                            # Deferred one head so the matmul never waits on
                            # the DVE esum chain.
                            rr = ps_pool.tile([128, NC], F32, tag="pv",
                                              name=f"rr{h}_{jc}")
                            nc.tensor.matmul(rr[:], ones_sk[:], esf[:],
                                             start=True, stop=True)
                            rbs = rinv_pool.tile([128, NC], F32, tag="rbs",
                                                 name=f"rbs{h}_{jc}")
                            nc.vector.reciprocal_approx_fast(rbs[:], rr[:])
                            ats = ats_pool.tile([Dh, NC], BF, tag="ats",
                                                name=f"ats{h}_{jc}")
                            nc.vector.tensor_mul(ats[:], pv[:], rbs[:])
                            ats_t.append(ats)

                        # out-projection chains are emitted in 4-matmul
                        # groups via this generator-like cursor so they can
                        # be interleaved into the score loop.
                        class DChain:
                            def __init__(self):
                                self.jobs = []  # (djc, tl, ncol)
                                self.pos = 0
                                self.ps = None
                                self.ats = None

                            def add(self, djc, tl, ncol, d_ats):
                                self.jobs.append((djc, tl, ncol, d_ats))

                            def emit(self, n):
                                # emit n matmuls worth of chain work
                                while n > 0 and (self.jobs or self.ps):
                                    if self.ps is None:
                                        djc, tl, ncol, d_ats = self.jobs.pop(0)
                                        self.cur = (djc, tl, ncol)
                                        self.ats = d_ats
                                        self.pos = 0
                                        self.ps = ps_pool.tile(
                                            [128, NC], F32, tag="pb",
                                            name=f"psD{djc}_{tl}_{ncol}")
                                    djc, tl, ncol = self.cur
                                    toff = slice(tl * 128, (tl + 1) * 128)
                                    cc = slice(ncol * NC, (ncol + 1) * NC)
                                    take = min(n, H - self.pos)
                                    for h in range(self.pos, self.pos + take):
                                        nc.tensor.matmul(
                                            self.ps[:], self.ats[h][:, toff],
                                            wo_t[h][:, cc],
                                            start=(h == 0), stop=(h == H - 1))
                                    self.pos += take
                                    n -= take
                                    if self.pos == H:
                                        tt = slice(djc * NC + tl * 128,
                                                   djc * NC + (tl + 1) * 128)
                                        st = outst_pool.tile(
                                            [128, NC], BF, tag="outst",
                                            name=f"outst{djc}_{tl}_{ncol}")
                                        nc.scalar.copy(st[:], self.ps[:])
                                        nc.sync.dma_start(out=out_d[tt, cc],
                                                          in_=st[:])
                                        self.ps = None

                        dchain = DChain()
                        prev_ats = None
                        for jc in range(JC):
                            jj = slice(jc * NC, (jc + 1) * NC)
                            ats_t = []
                            pending = None
                            for h in range(H):
                                if prev_ats is not None:
                                    for k in range(4):
                                        idx = h * 4 + k
                                        dchain.add(jc - 1, idx // ND,
                                                   idx % ND, prev_ats)
                                ets = []
                                for i2 in range(SK // 2):
                                    ps2 = ps_pool.tile(
                                        [128, 2 * NC], F32, tag="sc",
                                        name=f"sc{h}_{jc}_{i2}")
                                    for p in range(2):
                                        i = i2 * 2 + p
                                        nc.tensor.matmul(
                                            ps2[:, p * NC:(p + 1) * NC],
                                            kT_t[h][:, i * 128:(i + 1) * 128],
                                            qT_t[h][:, jj],
                                            start=True, stop=True)
                                    # 4 out-proj matmuls of the PREVIOUS
                                    # query chunk between score pairs: ready
                                    # PE work that absorbs the exp drain.
                                    dchain.emit(4)
                                    et = et_pool.tile([128, 2 * NC], BF,
                                                      tag="ET",
                                                      name=f"et{h}_{jc}_{i2}")
                                    nc.scalar.activation(et[:], ps2[:], Exp)
                                    ets.append(et)
                                # previous head's norm ops go FIRST so its
                                # ats never queues behind this head's esum
                                # chain on DVE (pv buffer reuse waits on ats)
                                if pending is not None:
                                    emit_norm(jc, h - 1, *pending, ats_t)
                                # DVE tree-sum of the exp tiles for the
                                # softmax denominator.
                                es = esum_pool.tile([128, 2 * NC], BF,
                                                    tag="es", name=f"es{h}_{jc}")
                                nc.vector.tensor_add(es[:], ets[0][:], ets[1][:])
                                for i2 in range(2, SK // 2):
                                    nc.vector.tensor_add(es[:], es[:], ets[i2][:])
                                esf = esum_pool.tile([128, NC], BF, tag="esf",
                                                     name=f"esf{h}_{jc}")
                                nc.vector.tensor_add(esf[:], es[:, 0:NC],
                                                     es[:, NC:2 * NC])
                                pv = ps_pool.tile([Dh, NC], F32, tag="pv",
                                                    name=f"pv{h}_{jc}")
                                for i2 in range(SK // 2):
                                    for p in range(2):
                                        i = i2 * 2 + p
                                        sl = ets[i2][:, p * NC:(p + 1) * NC]
                                        nc.tensor.matmul(
                                            pv[:],
                                            v_t[i][:, h * Dh:(h + 1) * Dh],
                                            sl, start=(i == 0),
                                            stop=(i == SK - 1))
                                pending = (esf, pv)
                            emit_norm(jc, H - 1, *pending, ats_t)
                            prev_ats = ats_t

                        # out-projection for the last query chunk
                        for tl in range(NC // 128):
                            for ncol in range(ND):
                                dchain.add(JC - 1, tl, ncol, prev_ats)
                        dchain.emit(10 ** 9)
                    ps_cm.__exit__(None, None, None)

    nc.compile()
    return nc


def host_inputs(hidden_states, Wq, Wc, Wk, Wv, Wo, S=SEQ, Dh=HEAD_DIM,
                heads_per_core=HEADS_PER_CORE, n_cores=N_CORES):
    """Shard + preprocess full fp32 inputs into per-core bf16 in_maps."""
    scale = 1.0 / np.sqrt(Dh)
    pos = np.arange(S, dtype=np.float32)
    inv_freq = 1.0 / (ROPE_THETA ** (np.arange(0, Dh, 2, dtype=np.float32) / Dh))
    freqs = pos[:, None] * inv_freq
    emb = np.concatenate([freqs, freqs], axis=-1)      # [S, Dh]
    cosT = np.cos(emb).T.copy()                        # [Dh, S]
    sinT = np.sin(emb).T.copy()
    sinT[: Dh // 2] *= -1.0                            # sign baked for the swap trick
    cosq = (cosT * scale).astype(BF_NP)
    sinq = (sinT * scale).astype(BF_NP)
    cosk = cosT.astype(BF_NP)
    sink = sinT.astype(BF_NP)

    hw = heads_per_core * Dh
    in_maps = []
    for c in range(n_cores):
        b, g = divmod(c, 4)
        cols = slice(g * hw, (g + 1) * hw)
        in_maps.append({
            "hsT": np.ascontiguousarray(hidden_states[b].T).astype(BF_NP),
            "Wq": np.ascontiguousarray(Wq[:, cols]).astype(BF_NP),
            "Wc": Wc.astype(BF_NP),
            "Wk": np.ascontiguousarray(Wk[:, cols]).astype(BF_NP),
            "Wv": np.ascontiguousarray(Wv[:, cols]).astype(BF_NP),
            "Wo": np.ascontiguousarray(Wo[cols, :]).astype(BF_NP),
            "cosq": cosq, "sinq": sinq, "cosk": cosk, "sink": sink,
        })
    return in_maps


_NC_CACHE = {}


def kernel(hidden_states, Wq, Wc, Wk, Wv, Wo):
    hidden_states = np.asarray(hidden_states, dtype=np.float32)
    if "nc" not in _NC_CACHE:
        _NC_CACHE["nc"] = build_nc()
    nc = _NC_CACHE["nc"]
    in_maps = host_inputs(hidden_states, np.asarray(Wq, np.float32),
                          np.asarray(Wc, np.float32), np.asarray(Wk, np.float32),
                          np.asarray(Wv, np.float32), np.asarray(Wo, np.float32))
    res = run_bass_kernel_spmd(nc, in_maps, list(range(N_CORES))).results
    B, S, D = BATCH, SEQ, D_MODEL
    out = np.zeros((B, S, D), dtype=np.float32)
    for c in range(N_CORES):
        out[c // 4] += res[c]["out"]
    return out


# revision 37
# speedup vs baseline: 1.0738x; 1.0041x over previous
"""Multi-head latent attention (MLA) Bass kernel for 8 TRN2 NeuronCores.

Sharding: tensor-parallel over heads x data-parallel over batch.
Core c (0..7) owns batch b = c//4 and head group g = c%4 (8 heads of 32).

No collectives: a NEFF containing any collective_compute runs the PE at
~263ns/512col instead of ~216ns (a global ~22% clock penalty), so each
core computes the FULL latent for its batch itself (+83us of matmul)
instead of sharding latent + AllGather. The latent stays resident in
SBUF between phase A and B (no DRAM round trip).

Phases (single core):
  A: per seq chunk j: latent chains (Wc) + q-proj chains (Wq) share the
     same hsT chunk tiles; q RoPE'd on DVE into qT [Dh, S] per head.
  B: kT (RoPE'd, [Dh, S]) + v from SBUF-resident latent; RoPE split
     across Scalar (kb copy), DVE (psum-reading muls) and GpSimd
     (bf16 mul + add) so no single engine paces PE; Wo prefetched.
  C: per query chunk: scores -> exp (ACT) -> DVE tree-sum denominator +
     ones-matmul partition reduction; attention fused with
     out-projection; out-proj chains of the previous chunk are
     interleaved INTO the score loop (4 mms per score slot) so PE never
     waits on the exp drain.
Host sums the 4 partials per batch.

Compute dtype: bf16 on the TensorE inputs, fp32 PSUM accumulation.
"""

import sys

for _p in ("/opt/trn_rl_repo", "/root/.axon_site/_ro/trn_rl_repo"):
    if _p not in sys.path:
        sys.path.insert(0, _p)

import numpy as np
import ml_dtypes

import concourse.bacc as bacc
import concourse.mybir as mybir
import concourse.tile as tile
from concourse.bass_utils import run_bass_kernel_spmd

BF = mybir.dt.bfloat16
F32 = mybir.dt.float32
BF_NP = ml_dtypes.bfloat16

# Full-problem constants (hardcoded per the self-contained-kernel contract).
D_MODEL = 4096
D_LATENT = 512
NUM_HEADS = 32
HEAD_DIM = 128
ROPE_THETA = 10000.0
BATCH, SEQ = 2, 2048
N_CORES = 8
HEADS_PER_CORE = NUM_HEADS // 4  # 4 head groups x 2 batches = 8 cores


def build_nc(S=SEQ, D=D_MODEL, L=D_LATENT, H=HEADS_PER_CORE, Dh=HEAD_DIM,
             NA=512, NC=512):
    """Build the single-core Bass program (SPMD across 8 cores)."""
    assert S % NA == 0 and S % 128 == 0 and D % 128 == 0 and L % 128 == 0
    KD = D // 128     # contraction chunks over d_model
    LD = L // 128     # contraction chunks over d_latent
    JA = S // NA      # seq chunks in projection phase
    JC = S // NC      # seq chunks in attention phase
    SK = S // 128     # key-position chunks
    HD1 = H * Dh      # this core's total head width (1024)
    ND = D // NC      # output-column chunks

    nc = bacc.Bacc("TRN2", target_bir_lowering=False)

    hsT_d = nc.declare_dram_parameter("hsT", [D, S], BF, isOutput=False)
    wq_d = nc.declare_dram_parameter("Wq", [D, HD1], BF, isOutput=False)
    wc_d = nc.declare_dram_parameter("Wc", [D, L], BF, isOutput=False)
    wk_d = nc.declare_dram_parameter("Wk", [L, HD1], BF, isOutput=False)
    wv_d = nc.declare_dram_parameter("Wv", [L, HD1], BF, isOutput=False)
    wo_d = nc.declare_dram_parameter("Wo", [HD1, D], BF, isOutput=False)
    cosq_d = nc.declare_dram_parameter("cosq", [Dh, S], BF, isOutput=False)
    sinq_d = nc.declare_dram_parameter("sinq", [Dh, S], BF, isOutput=False)
    cosk_d = nc.declare_dram_parameter("cosk", [Dh, S], BF, isOutput=False)
    sink_d = nc.declare_dram_parameter("sink", [Dh, S], BF, isOutput=False)
    out_d = nc.declare_dram_parameter("out", [S, D], BF, isOutput=True)
    latq_d = nc.dram_tensor("latq_dram", [L, S], BF)

    Exp = mybir.ActivationFunctionType.Exp
    half = Dh // 2

    with tile.TileContext(nc) as tc:
        with tc.tile_pool(name="consts", bufs=1) as const_pool:
            ones_sk = const_pool.tile([128, 128], BF)
            nc.vector.memset(ones_sk[:], 1.0)
            with tc.tile_pool(name="qT", bufs=1) as qT_pool:
                latB_cm = tc.tile_pool(name="latB", bufs=1, side="right")
                latB_pool = latB_cm.__enter__()
                wkv_cm = tc.tile_pool(name="wkv", bufs=1, side="right")
                wkv_pool = wkv_cm.__enter__()
                qT_t = [qT_pool.tile([Dh, S], BF, name=f"qT{h}") for h in range(H)]
                # Right-side stack: coexists with phase A's left-side pools,
                # so the k/v-expand inputs stream in DURING A instead of
                # serializing behind the pool-stack barrier at A's end.
                lq_t = [latB_pool.tile([128, S], BF, name=f"latB{ld}")
                        for ld in range(LD)]
                wk_t = [wkv_pool.tile([128, HD1], BF, name=f"wk{ld}")
                        for ld in range(LD)]
                wv_t = [wkv_pool.tile([128, HD1], BF, name=f"wv{ld}")
                        for ld in range(LD)]
                for ld in range(LD):
                    nc.gpsimd.dma_start(
                        out=wk_t[ld][:], in_=wk_d[ld * 128:(ld + 1) * 128, :])
                for ld in range(LD):
                    nc.gpsimd.dma_start(
                        out=wv_t[ld][:], in_=wv_d[ld * 128:(ld + 1) * 128, :])

                # == Phase A: full latent + qT (with RoPE), shared hs tiles ==
                with tc.tile_pool(name="wcA", bufs=1) as wcA_pool, \
                     tc.tile_pool(name="wqA", bufs=1) as wqA_pool, \
                     tc.tile_pool(name="hsA", bufs=KD + 1) as hsA_pool, \
                     tc.tile_pool(name="ropeq", bufs=1) as ropeq_pool, \
                     tc.tile_pool(name="tmpA", bufs=2) as tmpA_pool, \
                     tc.tile_pool(name="psA", bufs=8, space="PSUM") as psA_pool:

                    HW2 = HD1 // 2
                    wq_t = [[wqA_pool.tile([128, HW2], BF,
                                           name=f"wq{p}_{kd}")
                             for kd in range(KD)] for p in range(2)]
                    wc_t = [wcA_pool.tile([128, L], BF, name=f"wc{kd}")
                            for kd in range(KD)]
                    cosq_sb = ropeq_pool.tile([Dh, S], BF)
                    sinq_sb = ropeq_pool.tile([Dh, S], BF)

                    def emit_rope_q(j, h, ps):
                        jj = slice(j * NA, (j + 1) * NA)
                        t1 = tmpA_pool.tile([128, NA], BF, tag="t1",
                                            name=f"t1q{j}_{h}")
                        t2 = tmpA_pool.tile([128, NA], BF, tag="t2",
                                            name=f"t2q{j}_{h}")
                        nc.vector.tensor_mul(t1[:], ps[:], cosq_sb[:, jj])
                        nc.vector.tensor_mul(t2[0:half, :], ps[half:Dh, :],
                                             sinq_sb[0:half, jj])
                        nc.vector.tensor_mul(t2[half:Dh, :], ps[0:half, :],
                                             sinq_sb[half:Dh, jj])
                        nc.vector.tensor_add(qT_t[h][:, jj], t1[:], t2[:])

                    def emit_lsb(j, ps_l):
                        # latent -> DRAM, then straight back into the
                        # right-side latB tiles (both flow during A)
                        jj = slice(j * NA, (j + 1) * NA)
                        for ld in range(LD):
                            lsb = tmpA_pool.tile([128, NA], BF, tag="lsb",
                                                 name=f"lsb{j}_{ld}")
                            nc.scalar.copy(lsb[:], ps_l[ld][:])
                            nc.sync.dma_start(
                                out=latq_d[ld * 128:(ld + 1) * 128, jj],
                                in_=lsb[:])
                            nc.scalar.dma_start(
                                out=lq_t[ld][:, jj],
                                in_=latq_d[ld * 128:(ld + 1) * 128, jj])

                    # ---- j0: cold start is DMA-paced, so interleave the
                    # latent chains with q chains h0-3 kd-outer: per kd PE
                    # does 8 matmuls (1.7us) while the rings deliver the
                    # (wc, hs, wq_lo) triple for kd+1.
                    j0 = slice(0, NA)
                    hs_c0 = []
                    for kd in range(KD):
                        nc.sync.dma_start(
                            out=wc_t[kd][:],
                            in_=wc_d[kd * 128:(kd + 1) * 128, :])
                        t = hsA_pool.tile([128, NA], BF, tag="hsA",
                                          name=f"hsA_0_{kd}")
                        nc.scalar.dma_start(
                            out=t[:], in_=hsT_d[kd * 128:(kd + 1) * 128, j0])
                        hs_c0.append(t)
                        (nc.sync if kd % 2 == 0 else nc.scalar).dma_start(
                            out=wq_t[0][kd][:],
                            in_=wq_d[kd * 128:(kd + 1) * 128, 0:HW2])
                    nc.gpsimd.dma_start(out=cosq_sb[:], in_=cosq_d[:])
                    nc.gpsimd.dma_start(out=sinq_sb[:], in_=sinq_d[:])
                    for kd in range(KD):
                        nc.gpsimd.dma_start(
                            out=wq_t[1][kd][:],
                            in_=wq_d[kd * 128:(kd + 1) * 128, HW2:HD1])
                    ps_l0 = [psA_pool.tile([128, NA], F32, tag="psA",
                                           name=f"psL0_{ld}")
                             for ld in range(LD)]
                    ps_q0 = [psA_pool.tile([128, NA], F32, tag="psA",
                                           name=f"psQ0_{h}")
                             for h in range(4)]
                    for kd in range(KD):
                        for ld in range(LD):
                            nc.tensor.matmul(
                                ps_l0[ld][:],
                                wc_t[kd][:, ld * 128:(ld + 1) * 128],
                                hs_c0[kd][:],
                                start=(kd == 0), stop=(kd == KD - 1))
                        for h in range(4):
                            nc.tensor.matmul(
                                ps_q0[h][:],
                                wq_t[0][kd][:, h * Dh:h * Dh + Dh],
                                hs_c0[kd][:],
                                start=(kd == 0), stop=(kd == KD - 1))
                    emit_lsb(0, ps_l0)
                    for h in range(4):
                        emit_rope_q(0, h, ps_q0[h])
                    for h in range(4, H):
                        hp, hq = divmod(h * Dh, HW2)
                        ps = psA_pool.tile([128, NA], F32, tag="psA",
                                           name=f"psQ0_{h}")
                        for kd in range(KD):
                            nc.tensor.matmul(
                                ps[:], wq_t[hp][kd][:, hq:hq + Dh],
                                hs_c0[kd][:],
                                start=(kd == 0), stop=(kd == KD - 1))
                        emit_rope_q(0, h, ps)

                    # ---- j1..j3: steady state; hs chunks for j+1 prefetch
                    # freely during j (hsA ring holds two full j's).
                    for j in range(1, JA):
                        jj = slice(j * NA, (j + 1) * NA)
                        hs_ch = []
                        for kd in range(KD):
                            t = hsA_pool.tile([128, NA], BF, tag="hsA",
                                              name=f"hsA_{j}_{kd}")
                            (nc.sync if kd % 2 == 0 else nc.scalar).dma_start(
                                out=t[:], in_=hsT_d[kd * 128:(kd + 1) * 128, jj])
                            hs_ch.append(t)

                        # latent chains for this j (kd-outer, 4 chains)
                        ps_l = [psA_pool.tile([128, NA], F32, tag="psA",
                                              name=f"psL{j}_{ld}")
                                for ld in range(LD)]
                        for kd in range(KD):
                            for ld in range(LD):
                                nc.tensor.matmul(
                                    ps_l[ld][:],
                                    wc_t[kd][:, ld * 128:(ld + 1) * 128],
                                    hs_ch[kd][:],
                                    start=(kd == 0), stop=(kd == KD - 1))
                        emit_lsb(j, ps_l)

                        # q-proj chains + RoPE
                        for h in range(H):
                            hp, hq = divmod(h * Dh, HW2)
                            ps = psA_pool.tile([128, NA], F32, tag="psA",
                                               name=f"psQ{j}_{h}")
                            for kd in range(KD):
                                nc.tensor.matmul(
                                    ps[:], wq_t[hp][kd][:, hq:hq + Dh],
                                    hs_ch[kd][:],
                                    start=(kd == 0), stop=(kd == KD - 1))
                            emit_rope_q(j, h, ps)

                # ========== Phase B: kT (with RoPE) + v; prefetch Wo ========
                with tc.tile_pool(name="kT", bufs=1) as kT_pool, \
                     tc.tile_pool(name="v", bufs=1) as v_pool, \
                     tc.tile_pool(name="wo", bufs=1) as wo_pool:
                    kT_t = [kT_pool.tile([Dh, S], BF, name=f"kT{h}")
                            for h in range(H)]
                    v_t = [v_pool.tile([128, HD1], BF, name=f"v{i}")
                           for i in range(SK)]
                    wo_t = [wo_pool.tile([128, D], BF, name=f"wo{h}")
                            for h in range(H)]

                    # One PSUM pool spans B and C' so C's score banks are
                    # disjoint from B's working banks (no WAR delay):
                    # tags: pb 2x1 bank (B kT/v psums + C' out-proj),
                    #       sc 2x2 banks (scores), pv 2x1 bank (pv + denom).
                    ps_cm = tc.tile_pool(name="pswork", bufs=2, space="PSUM")
                    ps_pool = ps_cm.__enter__()
                    with tc.tile_pool(name="ropek", bufs=1) as ropek_pool, \
                         tc.tile_pool(name="tmpB", bufs=1) as tmpB_pool:

                        cosk_sb = ropek_pool.tile([Dh, S], BF)
                        sink_sb = ropek_pool.tile([Dh, S], BF)
                        nc.scalar.dma_start(out=cosk_sb[:], in_=cosk_d[:])
                        nc.scalar.dma_start(out=sink_sb[:], in_=sink_d[:])
                        for ld in range(LD):
                            nc.sync.dma_start(
                                out=wv_t[ld][:],
                                in_=wv_d[ld * 128:(ld + 1) * 128, :])
                        # Wo prefetch early: B has ~60us, Wo is 8MB over two
                        # queues; first consumer is C' jc=1 (~70us later).
                        for hh in range(H):
                            eng = nc.sync if hh % 2 == 0 else nc.scalar
                            eng.dma_start(
                                out=wo_t[hh][:],
                                in_=wo_d[hh * 128:(hh + 1) * 128, :])

                        # kT: h outer so each head's kT completes early and
                        # unblocks that head's score matmuls in C'. Blocks
                        # are 1024 wide on the idle "sc" psum banks to halve
                        # per-op overheads. RoPE work is split: kb copy on
                        # Scalar, psum-reading muls + t1 on DVE, final add
                        # on GpSimd -- no single engine paces PE. v-expand
                        # chains (from SBUF-resident lq) interleave to keep
                        # PE dense.
                        NB = 1024
                        for h in range(H):
                            for j in range(S // NB):
                                jj = slice(j * NB, (j + 1) * NB)
                                ps = ps_pool.tile([128, NB], F32, tag="sc",
                                                  name=f"psK{h}_{j}")
                                for p in range(2):
                                    pp = slice(p * 512, (p + 1) * 512)
                                    jp = slice(j * NB + p * 512,
                                               j * NB + (p + 1) * 512)
                                    for ld in range(LD):
                                        nc.tensor.matmul(
                                            ps[:, pp],
                                            wk_t[ld][:, h * Dh:(h + 1) * Dh],
                                            lq_t[ld][:, jp],
                                            start=(ld == 0),
                                            stop=(ld == LD - 1))
                                kb = tmpB_pool.tile([128, NB], BF, tag="kb",
                                                    name=f"kb{h}_{j}")
                                nc.scalar.copy(kb[:], ps[:])
                                t1 = tmpB_pool.tile([128, NB], BF, tag="t1b",
                                                    name=f"t1k{h}_{j}")
                                t2 = tmpB_pool.tile([128, NB], BF, tag="t2b",
                                                    name=f"t2k{h}_{j}")
                                nc.vector.tensor_mul(t2[0:half, :],
                                                     ps[half:Dh, :],
                                                     sink_sb[0:half, jj])
                                nc.vector.tensor_mul(t2[half:Dh, :],
                                                     ps[0:half, :],
                                                     sink_sb[half:Dh, jj])
                                nc.vector.tensor_mul(t1[:], kb[:],
                                                     cosk_sb[:, jj])
                                nc.gpsimd.tensor_add(kT_t[h][:, jj],
                                                     t1[:], t2[:])
                                # one v seq-tile after each double-block
                                i = 2 * h + j
                                io = slice(i * 128, (i + 1) * 128)
                                for cch in range(HD1 // 512):
                                    cc = slice(cch * 512, (cch + 1) * 512)
                                    ps = ps_pool.tile([128, 512], F32,
                                                      tag="pb",
                                                      name=f"psV{i}_{cch}")
                                    for ld in range(LD):
                                        nc.tensor.matmul(
                                            ps[:], lq_t[ld][:, io],
                                            wv_t[ld][:, cc],
                                            start=(ld == 0),
                                            stop=(ld == LD - 1))
                                    nc.scalar.copy(v_t[i][:, cc], ps[:])

                    wkv_cm.__exit__(None, None, None)
                    latB_cm.__exit__(None, None, None)

                    # ===== Phase C': attention fused with out-projection =====
                    with tc.tile_pool(name="ET", bufs=8) as et_pool, \
                         tc.tile_pool(name="esum", bufs=1) as esum_pool, \
                         tc.tile_pool(name="rinv", bufs=1) as rinv_pool, \
                         tc.tile_pool(name="ats", bufs=2 * H) as ats_pool, \
                         tc.tile_pool(name="outst", bufs=2) as outst_pool:

                        assert SK % 2 == 0

                        def emit_norm(jc, h, esf, pv, ats_t):
                            # denominator matmul + reciprocal + normalize.
                            # Deferred one head so the matmul never waits on
                            # the DVE esum chain.
                            rr = ps_pool.tile([128, NC], F32, tag="pv",
                                              name=f"rr{h}_{jc}")
                            nc.tensor.matmul(rr[:], ones_sk[:], esf[:],
                                             start=True, stop=True)
                            rbs = rinv_pool.tile([128, NC], F32, tag="rbs",
                                                 name=f"rbs{h}_{jc}")
                            nc.vector.reciprocal_approx_fast(rbs[:], rr[:])
                            ats = ats_pool.tile([Dh, NC], BF, tag="ats",
                                                name=f"ats{h}_{jc}")
                            nc.vector.tensor_mul(ats[:], pv[:], rbs[:])
                            ats_t.append(ats)

                        # out-projection chains are emitted in 4-matmul
                        # groups via this generator-like cursor so they can
                        # be interleaved into the score loop.
                        class DChain:
                            def __init__(self):
                                self.jobs = []  # (djc, tl, ncol)
                                self.pos = 0
                                self.ps = None
                                self.ats = None

                            def add(self, djc, tl, ncol, d_ats):
                                self.jobs.append((djc, tl, ncol, d_ats))

                            def emit(self, n):
                                # emit n matmuls worth of chain work
                                while n > 0 and (self.jobs or self.ps):
                                    if self.ps is None:
                                        djc, tl, ncol, d_ats = self.jobs.pop(0)
                                        self.cur = (djc, tl, ncol)
                                        self.ats = d_ats
                                        self.pos = 0
                                        self.ps = ps_pool.tile(
                                            [128, NC], F32, tag="pb",
                                            name=f"psD{djc}_{tl}_{ncol}")
                                    djc, tl, ncol = self.cur
                                    toff = slice(tl * 128, (tl + 1) * 128)
                                    cc = slice(ncol * NC, (ncol + 1) * NC)
                                    take = min(n, H - self.pos)
                                    for h in range(self.pos, self.pos + take):
                                        nc.tensor.matmul(
                                            self.ps[:], self.ats[h][:, toff],
                                            wo_t[h][:, cc],
                                            start=(h == 0), stop=(h == H - 1))
                                    self.pos += take
                                    n -= take
                                    if self.pos == H:
                                        tt = slice(djc * NC + tl * 128,
                                                   djc * NC + (tl + 1) * 128)
                                        st = outst_pool.tile(
                                            [128, NC], BF, tag="outst",
                                            name=f"outst{djc}_{tl}_{ncol}")
                                        nc.scalar.copy(st[:], self.ps[:])
                                        nc.sync.dma_start(out=out_d[tt, cc],
                                                          in_=st[:])
                                        self.ps = None

                        dchain = DChain()
                        prev_ats = None
                        for jc in range(JC):
                            jj = slice(jc * NC, (jc + 1) * NC)
                            ats_t = []
                            pending = None
                            for h in range(H):
                                if prev_ats is not None:
                                    for k in range(4):
                                        idx = h * 4 + k
                                        dchain.add(jc - 1, idx // ND,
                                                   idx % ND, prev_ats)
                                ets = []
                                for i2 in range(SK // 2):
                                    ps2 = ps_pool.tile(
                                        [128, 2 * NC], F32, tag="sc",
                                        name=f"sc{h}_{jc}_{i2}")
                                    for p in range(2):
                                        i = i2 * 2 + p
                                        nc.tensor.matmul(
                                            ps2[:, p * NC:(p + 1) * NC],
                                            kT_t[h][:, i * 128:(i + 1) * 128],
                                            qT_t[h][:, jj],
                                            start=True, stop=True)
                                    # 4 out-proj matmuls of the PREVIOUS
                                    # query chunk between score pairs: ready
                                    # PE work that absorbs the exp drain.
                                    dchain.emit(4)
                                    et = et_pool.tile([128, 2 * NC], BF,
                                                      tag="ET",
                                                      name=f"et{h}_{jc}_{i2}")
                                    nc.scalar.activation(et[:], ps2[:], Exp)
                                    ets.append(et)
                                # previous head's norm ops go FIRST so its
                                # ats never queues behind this head's esum
                                # chain on DVE (pv buffer reuse waits on ats)
                                if pending is not None:
                                    emit_norm(jc, h - 1, *pending, ats_t)
                                # DVE tree-sum of the exp tiles for the
                                # softmax denominator.
                                es = esum_pool.tile([128, 2 * NC], BF,
                                                    tag="es", name=f"es{h}_{jc}")
                                nc.vector.tensor_add(es[:], ets[0][:], ets[1][:])
                                for i2 in range(2, SK // 2):
                                    nc.vector.tensor_add(es[:], es[:], ets[i2][:])
                                esf = esum_pool.tile([128, NC], BF, tag="esf",
                                                     name=f"esf{h}_{jc}")
                                nc.vector.tensor_add(esf[:], es[:, 0:NC],
                                                     es[:, NC:2 * NC])
                                pv = ps_pool.tile([Dh, NC], F32, tag="pv",
                                                    name=f"pv{h}_{jc}")
                                for i2 in range(SK // 2):
                                    for p in range(2):
                                        i = i2 * 2 + p
                                        sl = ets[i2][:, p * NC:(p + 1) * NC]
                                        nc.tensor.matmul(
                                            pv[:],
                                            v_t[i][:, h * Dh:(h + 1) * Dh],
                                            sl, start=(i == 0),
                                            stop=(i == SK - 1))
                                pending = (esf, pv)
                            emit_norm(jc, H - 1, *pending, ats_t)
                            prev_ats = ats_t

                        # out-projection for the last query chunk
                        for tl in range(NC // 128):
                            for ncol in range(ND):
                                dchain.add(JC - 1, tl, ncol, prev_ats)
                        dchain.emit(10 ** 9)
                    ps_cm.__exit__(None, None, None)

    nc.compile()
    return nc


def host_inputs(hidden_states, Wq, Wc, Wk, Wv, Wo, S=SEQ, Dh=HEAD_DIM,
                heads_per_core=HEADS_PER_CORE, n_cores=N_CORES):
    """Shard + preprocess full fp32 inputs into per-core bf16 in_maps."""
    scale = 1.0 / np.sqrt(Dh)
    pos = np.arange(S, dtype=np.float32)
    inv_freq = 1.0 / (ROPE_THETA ** (np.arange(0, Dh, 2, dtype=np.float32) / Dh))
    freqs = pos[:, None] * inv_freq
    emb = np.concatenate([freqs, freqs], axis=-1)      # [S, Dh]
    cosT = np.cos(emb).T.copy()                        # [Dh, S]
    sinT = np.sin(emb).T.copy()
    sinT[: Dh // 2] *= -1.0                            # sign baked for the swap trick
    cosq = (cosT * scale).astype(BF_NP)
    sinq = (sinT * scale).astype(BF_NP)
    cosk = cosT.astype(BF_NP)
    sink = sinT.astype(BF_NP)

    hw = heads_per_core * Dh
    in_maps = []
    for c in range(n_cores):
        b, g = divmod(c, 4)
        cols = slice(g * hw, (g + 1) * hw)
        in_maps.append({
            "hsT": np.ascontiguousarray(hidden_states[b].T).astype(BF_NP),
            "Wq": np.ascontiguousarray(Wq[:, cols]).astype(BF_NP),
            "Wc": Wc.astype(BF_NP),
            "Wk": np.ascontiguousarray(Wk[:, cols]).astype(BF_NP),
            "Wv": np.ascontiguousarray(Wv[:, cols]).astype(BF_NP),
            "Wo": np.ascontiguousarray(Wo[cols, :]).astype(BF_NP),
            "cosq": cosq, "sinq": sinq, "cosk": cosk, "sink": sink,
        })
    return in_maps


_NC_CACHE = {}


def kernel(hidden_states, Wq, Wc, Wk, Wv, Wo):
    hidden_states = np.asarray(hidden_states, dtype=np.float32)
    if "nc" not in _NC_CACHE:
        _NC_CACHE["nc"] = build_nc()
    nc = _NC_CACHE["nc"]
    in_maps = host_inputs(hidden_states, np.asarray(Wq, np.float32),
                          np.asarray(Wc, np.float32), np.asarray(Wk, np.float32),
                          np.asarray(Wv, np.float32), np.asarray(Wo, np.float32))
    res = run_bass_kernel_spmd(nc, in_maps, list(range(N_CORES))).results
    B, S, D = BATCH, SEQ, D_MODEL
    out = np.zeros((B, S, D), dtype=np.float32)
    for c in range(N_CORES):
        out[c // 4] += res[c]["out"]
    return out


# revision 40
# speedup vs baseline: 1.0767x; 1.0027x over previous
"""Multi-head latent attention (MLA) Bass kernel for 8 TRN2 NeuronCores.

Sharding: tensor-parallel over heads x data-parallel over batch.
Core c (0..7) owns batch b = c//4 and head group g = c%4 (8 heads of 32).

No collectives: a NEFF containing any collective_compute runs the PE at
~263ns/512col instead of ~216ns (a global ~22% clock penalty), so each
core computes the FULL latent for its batch itself (+83us of matmul)
instead of sharding latent + AllGather. The latent stays resident in
SBUF between phase A and B (no DRAM round trip).

Phases (single core):
  A: per seq chunk j: latent chains (Wc) + q-proj chains (Wq) share the
     same hsT chunk tiles; q RoPE'd on DVE into qT [Dh, S] per head.
  B: kT (RoPE'd, [Dh, S]) + v from SBUF-resident latent; RoPE split
     across Scalar (kb copy), DVE (psum-reading muls) and GpSimd
     (bf16 mul + add) so no single engine paces PE; Wo prefetched.
  C: per query chunk: scores -> exp (ACT) -> DVE tree-sum denominator +
     ones-matmul partition reduction; attention fused with
     out-projection; out-proj chains of the previous chunk are
     interleaved INTO the score loop (4 mms per score slot) so PE never
     waits on the exp drain.
Host sums the 4 partials per batch.

Compute dtype: bf16 on the TensorE inputs, fp32 PSUM accumulation.
"""

import sys

for _p in ("/opt/trn_rl_repo", "/root/.axon_site/_ro/trn_rl_repo"):
    if _p not in sys.path:
        sys.path.insert(0, _p)

import numpy as np
import ml_dtypes

import concourse.bacc as bacc
import concourse.mybir as mybir
import concourse.tile as tile
from concourse.bass_utils import run_bass_kernel_spmd

BF = mybir.dt.bfloat16
F32 = mybir.dt.float32
BF_NP = ml_dtypes.bfloat16

# Full-problem constants (hardcoded per the self-contained-kernel contract).
D_MODEL = 4096
D_LATENT = 512
NUM_HEADS = 32
HEAD_DIM = 128
ROPE_THETA = 10000.0
BATCH, SEQ = 2, 2048
N_CORES = 8
HEADS_PER_CORE = NUM_HEADS // 4  # 4 head groups x 2 batches = 8 cores


def build_nc(S=SEQ, D=D_MODEL, L=D_LATENT, H=HEADS_PER_CORE, Dh=HEAD_DIM,
             NA=512, NC=512):
    """Build the single-core Bass program (SPMD across 8 cores)."""
    assert S % NA == 0 and S % 128 == 0 and D % 128 == 0 and L % 128 == 0
    KD = D // 128     # contraction chunks over d_model
    LD = L // 128     # contraction chunks over d_latent
    JA = S // NA      # seq chunks in projection phase
    JC = S // NC      # seq chunks in attention phase
    SK = S // 128     # key-position chunks
    HD1 = H * Dh      # this core's total head width (1024)
    ND = D // NC      # output-column chunks

    nc = bacc.Bacc("TRN2", target_bir_lowering=False)

    hsT_d = nc.declare_dram_parameter("hsT", [D, S], BF, isOutput=False)
    wq_d = nc.declare_dram_parameter("Wq", [D, HD1], BF, isOutput=False)
    wc_d = nc.declare_dram_parameter("Wc", [D, L], BF, isOutput=False)
    wk_d = nc.declare_dram_parameter("Wk", [L, HD1], BF, isOutput=False)
    wv_d = nc.declare_dram_parameter("Wv", [L, HD1], BF, isOutput=False)
    wo_d = nc.declare_dram_parameter("Wo", [HD1, D], BF, isOutput=False)
    cosq_d = nc.declare_dram_parameter("cosq", [Dh, S], BF, isOutput=False)
    sinq_d = nc.declare_dram_parameter("sinq", [Dh, S], BF, isOutput=False)
    cosk_d = nc.declare_dram_parameter("cosk", [Dh, S], BF, isOutput=False)
    sink_d = nc.declare_dram_parameter("sink", [Dh, S], BF, isOutput=False)
    out_d = nc.declare_dram_parameter("out", [S, D], BF, isOutput=True)
    latq_d = nc.dram_tensor("latq_dram", [L, S], BF)

    Exp = mybir.ActivationFunctionType.Exp
    half = Dh // 2

    with tile.TileContext(nc) as tc:
        with tc.tile_pool(name="consts", bufs=1) as const_pool:
            ones_sk = const_pool.tile([128, 128], BF)
            nc.vector.memset(ones_sk[:], 1.0)
            with tc.tile_pool(name="qT", bufs=1) as qT_pool:
                latB_cm = tc.tile_pool(name="latB", bufs=1, side="right")
                latB_pool = latB_cm.__enter__()
                wkv_cm = tc.tile_pool(name="wkv", bufs=1, side="right")
                wkv_pool = wkv_cm.__enter__()
                qT_t = [qT_pool.tile([Dh, S], BF, name=f"qT{h}") for h in range(H)]
                # Right-side stack: coexists with phase A's left-side pools,
                # so the k/v-expand inputs stream in DURING A instead of
                # serializing behind the pool-stack barrier at A's end.
                lq_t = [latB_pool.tile([128, S], BF, name=f"latB{ld}")
                        for ld in range(LD)]
                wk_t = [wkv_pool.tile([128, HD1], BF, name=f"wk{ld}")
                        for ld in range(LD)]
                wv_t = [wkv_pool.tile([128, HD1], BF, name=f"wv{ld}")
                        for ld in range(LD)]
                for ld in range(LD):
                    nc.gpsimd.dma_start(
                        out=wk_t[ld][:], in_=wk_d[ld * 128:(ld + 1) * 128, :])
                for ld in range(LD):
                    nc.gpsimd.dma_start(
                        out=wv_t[ld][:], in_=wv_d[ld * 128:(ld + 1) * 128, :])

                # == Phase A: full latent + qT (with RoPE), shared hs tiles ==
                with tc.tile_pool(name="wcA", bufs=1) as wcA_pool, \
                     tc.tile_pool(name="wqA", bufs=1) as wqA_pool, \
                     tc.tile_pool(name="hsA", bufs=KD + 1) as hsA_pool, \
                     tc.tile_pool(name="ropeq", bufs=1) as ropeq_pool, \
                     tc.tile_pool(name="tmpA", bufs=2) as tmpA_pool, \
                     tc.tile_pool(name="psA", bufs=8, space="PSUM") as psA_pool:

                    HW2 = HD1 // 2
                    wq_t = [[wqA_pool.tile([128, HW2], BF,
                                           name=f"wq{p}_{kd}")
                             for kd in range(KD)] for p in range(2)]
                    wc_t = [wcA_pool.tile([128, L], BF, name=f"wc{kd}")
                            for kd in range(KD)]
                    cosq_sb = ropeq_pool.tile([Dh, S], BF)
                    sinq_sb = ropeq_pool.tile([Dh, S], BF)

                    def emit_rope_q(j, h, ps):
                        jj = slice(j * NA, (j + 1) * NA)
                        t1 = tmpA_pool.tile([128, NA], BF, tag="t1",
                                            name=f"t1q{j}_{h}")
                        t2 = tmpA_pool.tile([128, NA], BF, tag="t2",
                                            name=f"t2q{j}_{h}")
                        nc.vector.tensor_mul(t1[:], ps[:], cosq_sb[:, jj])
                        nc.vector.tensor_mul(t2[0:half, :], ps[half:Dh, :],
                                             sinq_sb[0:half, jj])
                        nc.vector.tensor_mul(t2[half:Dh, :], ps[0:half, :],
                                             sinq_sb[half:Dh, jj])
                        nc.vector.tensor_add(qT_t[h][:, jj], t1[:], t2[:])

                    def emit_lsb(j, ps_l):
                        # latent -> DRAM, then straight back into the
                        # right-side latB tiles (both flow during A)
                        jj = slice(j * NA, (j + 1) * NA)
                        for ld in range(LD):
                            lsb = tmpA_pool.tile([128, NA], BF, tag="lsb",
                                                 name=f"lsb{j}_{ld}")
                            nc.scalar.copy(lsb[:], ps_l[ld][:])
                            nc.sync.dma_start(
                                out=latq_d[ld * 128:(ld + 1) * 128, jj],
                                in_=lsb[:])
                            nc.scalar.dma_start(
                                out=lq_t[ld][:, jj],
                                in_=latq_d[ld * 128:(ld + 1) * 128, jj])

                    # ---- j0: cold start is DMA-paced, so interleave the
                    # latent chains with q chains h0-3 kd-outer: per kd PE
                    # does 8 matmuls (1.7us) while the rings deliver the
                    # (wc, hs, wq_lo) triple for kd+1.
                    j0 = slice(0, NA)
                    hs_c0 = []
                    for kd in range(KD):
                        nc.sync.dma_start(
                            out=wc_t[kd][:],
                            in_=wc_d[kd * 128:(kd + 1) * 128, :])
                        t = hsA_pool.tile([128, NA], BF, tag="hsA",
                                          name=f"hsA_0_{kd}")
                        nc.scalar.dma_start(
                            out=t[:], in_=hsT_d[kd * 128:(kd + 1) * 128, j0])
                        hs_c0.append(t)
                        (nc.sync if kd % 2 == 0 else nc.scalar).dma_start(
                            out=wq_t[0][kd][:],
                            in_=wq_d[kd * 128:(kd + 1) * 128, 0:HW2])
                    nc.gpsimd.dma_start(out=cosq_sb[:], in_=cosq_d[:])
                    nc.gpsimd.dma_start(out=sinq_sb[:], in_=sinq_d[:])
                    for kd in range(KD):
                        nc.gpsimd.dma_start(
                            out=wq_t[1][kd][:],
                            in_=wq_d[kd * 128:(kd + 1) * 128, HW2:HD1])
                    ps_l0 = [psA_pool.tile([128, NA], F32, tag="psA",
                                           name=f"psL0_{ld}")
                             for ld in range(LD)]
                    ps_q0 = [psA_pool.tile([128, NA], F32, tag="psA",
                                           name=f"psQ0_{h}")
                             for h in range(4)]
                    for kd in range(KD):
                        for ld in range(LD):
                            nc.tensor.matmul(
                                ps_l0[ld][:],
                                wc_t[kd][:, ld * 128:(ld + 1) * 128],
                                hs_c0[kd][:],
                                start=(kd == 0), stop=(kd == KD - 1))
                        for h in range(4):
                            nc.tensor.matmul(
                                ps_q0[h][:],
                                wq_t[0][kd][:, h * Dh:h * Dh + Dh],
                                hs_c0[kd][:],
                                start=(kd == 0), stop=(kd == KD - 1))
                    emit_lsb(0, ps_l0)
                    for h in range(4):
                        emit_rope_q(0, h, ps_q0[h])
                    for h in range(4, H):
                        hp, hq = divmod(h * Dh, HW2)
                        ps = psA_pool.tile([128, NA], F32, tag="psA",
                                           name=f"psQ0_{h}")
                        for kd in range(KD):
                            nc.tensor.matmul(
                                ps[:], wq_t[hp][kd][:, hq:hq + Dh],
                                hs_c0[kd][:],
                                start=(kd == 0), stop=(kd == KD - 1))
                        emit_rope_q(0, h, ps)

                    # ---- j1..j3: steady state; hs chunks for j+1 prefetch
                    # freely during j (hsA ring holds two full j's).
                    for j in range(1, JA):
                        jj = slice(j * NA, (j + 1) * NA)
                        hs_ch = []
                        for kd in range(KD):
                            t = hsA_pool.tile([128, NA], BF, tag="hsA",
                                              name=f"hsA_{j}_{kd}")
                            (nc.sync if kd % 2 == 0 else nc.scalar).dma_start(
                                out=t[:], in_=hsT_d[kd * 128:(kd + 1) * 128, jj])
                            hs_ch.append(t)

                        # latent chains for this j (kd-outer, 4 chains)
                        ps_l = [psA_pool.tile([128, NA], F32, tag="psA",
                                              name=f"psL{j}_{ld}")
                                for ld in range(LD)]
                        for kd in range(KD):
                            for ld in range(LD):
                                nc.tensor.matmul(
                                    ps_l[ld][:],
                                    wc_t[kd][:, ld * 128:(ld + 1) * 128],
                                    hs_ch[kd][:],
                                    start=(kd == 0), stop=(kd == KD - 1))
                        emit_lsb(j, ps_l)

                        # q-proj chains + RoPE
                        for h in range(H):
                            hp, hq = divmod(h * Dh, HW2)
                            ps = psA_pool.tile([128, NA], F32, tag="psA",
                                               name=f"psQ{j}_{h}")
                            for kd in range(KD):
                                nc.tensor.matmul(
                                    ps[:], wq_t[hp][kd][:, hq:hq + Dh],
                                    hs_ch[kd][:],
                                    start=(kd == 0), stop=(kd == KD - 1))
                            emit_rope_q(j, h, ps)

                # ========== Phase B: kT (with RoPE) + v; prefetch Wo ========
                with tc.tile_pool(name="kT", bufs=1) as kT_pool, \
                     tc.tile_pool(name="v", bufs=1) as v_pool, \
                     tc.tile_pool(name="wo", bufs=1) as wo_pool:
                    kT_t = [kT_pool.tile([Dh, S], BF, name=f"kT{h}")
                            for h in range(H)]
                    v_t = [v_pool.tile([128, HD1], BF, name=f"v{i}")
                           for i in range(SK)]
                    wo_t = [wo_pool.tile([128, D], BF, name=f"wo{h}")
                            for h in range(H)]

                    # One PSUM pool spans B and C' so C's score banks are
                    # disjoint from B's working banks (no WAR delay):
                    # tags: pb 2x1 bank (B kT/v psums + C' out-proj),
                    #       sc 2x2 banks (scores), pv 2x1 bank (pv + denom).
                    ps_cm = tc.tile_pool(name="pswork", bufs=2, space="PSUM")
                    ps_pool = ps_cm.__enter__()
                    with tc.tile_pool(name="ropek", bufs=1) as ropek_pool, \
                         tc.tile_pool(name="tmpB", bufs=1) as tmpB_pool:

                        cosk_sb = ropek_pool.tile([Dh, S], BF)
                        sink_sb = ropek_pool.tile([Dh, S], BF)
                        nc.scalar.dma_start(out=cosk_sb[:], in_=cosk_d[:])
                        nc.scalar.dma_start(out=sink_sb[:], in_=sink_d[:])
                        for ld in range(LD):
                            nc.sync.dma_start(
                                out=wv_t[ld][:],
                                in_=wv_d[ld * 128:(ld + 1) * 128, :])
                        # Wo prefetch early: B has ~60us, Wo is 8MB over two
                        # queues; first consumer is C' jc=1 (~70us later).
                        for hh in range(H):
                            eng = nc.sync if hh % 2 == 0 else nc.scalar
                            eng.dma_start(
                                out=wo_t[hh][:],
                                in_=wo_d[hh * 128:(hh + 1) * 128, :])

                        # kT: h outer so each head's kT completes early and
                        # unblocks that head's score matmuls in C'. Blocks
                        # are 1024 wide on the idle "sc" psum banks to halve
                        # per-op overheads. RoPE work is split: kb copy on
                        # Scalar, psum-reading muls + t1 on DVE, final add
                        # on GpSimd -- no single engine paces PE. v-expand
                        # chains (from SBUF-resident lq) interleave to keep
                        # PE dense.
                        NB = 1024
                        for h in range(H):
                            for j in range(S // NB):
                                jj = slice(j * NB, (j + 1) * NB)
                                ps = ps_pool.tile([128, NB], F32, tag="sc",
                                                  name=f"psK{h}_{j}")
                                for p in range(2):
                                    pp = slice(p * 512, (p + 1) * 512)
                                    jp = slice(j * NB + p * 512,
                                               j * NB + (p + 1) * 512)
                                    for ld in range(LD):
                                        nc.tensor.matmul(
                                            ps[:, pp],
                                            wk_t[ld][:, h * Dh:(h + 1) * Dh],
                                            lq_t[ld][:, jp],
                                            start=(ld == 0),
                                            stop=(ld == LD - 1))
                                kb = tmpB_pool.tile([128, NB], BF, tag="kb",
                                                    name=f"kb{h}_{j}")
                                nc.scalar.copy(kb[:], ps[:])
                                t1 = tmpB_pool.tile([128, NB], BF, tag="t1b",
                                                    name=f"t1k{h}_{j}")
                                t2 = tmpB_pool.tile([128, NB], BF, tag="t2b",
                                                    name=f"t2k{h}_{j}")
                                nc.vector.tensor_mul(t2[0:half, :],
                                                     ps[half:Dh, :],
                                                     sink_sb[0:half, jj])
                                nc.vector.tensor_mul(t2[half:Dh, :],
                                                     ps[0:half, :],
                                                     sink_sb[half:Dh, jj])
                                nc.vector.tensor_mul(t1[:], kb[:],
                                                     cosk_sb[:, jj])
                                nc.gpsimd.tensor_add(kT_t[h][:, jj],
                                                     t1[:], t2[:])
                                # one v seq-tile after each double-block
                                i = 2 * h + j
                                io = slice(i * 128, (i + 1) * 128)
                                for cch in range(HD1 // 512):
                                    cc = slice(cch * 512, (cch + 1) * 512)
                                    ps = ps_pool.tile([128, 512], F32,
                                                      tag="pb",
                                                      name=f"psV{i}_{cch}")
                                    for ld in range(LD):
                                        nc.tensor.matmul(
                                            ps[:], lq_t[ld][:, io],
                                            wv_t[ld][:, cc],
                                            start=(ld == 0),
                                            stop=(ld == LD - 1))
                                    if cch == 0:
                                        nc.scalar.copy(v_t[i][:, cc], ps[:])
                                    else:
                                        nc.vector.tensor_copy(
                                            v_t[i][:, cc], ps[:])

                    wkv_cm.__exit__(None, None, None)
                    latB_cm.__exit__(None, None, None)

                    # ===== Phase C': attention fused with out-projection =====
                    with tc.tile_pool(name="ET", bufs=8) as et_pool, \
                         tc.tile_pool(name="esum", bufs=1) as esum_pool, \
                         tc.tile_pool(name="rinv", bufs=1) as rinv_pool, \
                         tc.tile_pool(name="ats", bufs=2 * H) as ats_pool, \
                         tc.tile_pool(name="outst", bufs=2) as outst_pool:

                        assert SK % 2 == 0

                        def emit_norm(jc, h, esf, pv, ats_t):
                            # denominator matmul + reciprocal + normalize.
                            # Deferred one head so the matmul never waits on
                            # the DVE esum chain.
                            rr = ps_pool.tile([128, NC], F32, tag="pv",
                                              name=f"rr{h}_{jc}")
                            nc.tensor.matmul(rr[:], ones_sk[:], esf[:],
                                             start=True, stop=True)
                            rbs = rinv_pool.tile([128, NC], F32, tag="rbs",
                                                 name=f"rbs{h}_{jc}")
                            nc.vector.reciprocal_approx_fast(rbs[:], rr[:])
                            ats = ats_pool.tile([Dh, NC], BF, tag="ats",
                                                name=f"ats{h}_{jc}")
                            nc.vector.tensor_mul(ats[:], pv[:], rbs[:])
                            ats_t.append(ats)

                        # out-projection chains are emitted in 4-matmul
                        # groups via this generator-like cursor so they can
                        # be interleaved into the score loop.
                        class DChain:
                            def __init__(self):
                                self.jobs = []  # (djc, tl, ncol)
                                self.pos = 0
                                self.ps = None
                                self.ats = None

                            def add(self, djc, tl, ncol, d_ats):
                                self.jobs.append((djc, tl, ncol, d_ats))

                            def emit(self, n):
                                # emit n matmuls worth of chain work
                                while n > 0 and (self.jobs or self.ps):
                                    if self.ps is None:
                                        djc, tl, ncol, d_ats = self.jobs.pop(0)
                                        self.cur = (djc, tl, ncol)
                                        self.ats = d_ats
                                        self.pos = 0
                                        self.ps = ps_pool.tile(
                                            [128, NC], F32, tag="pb",
                                            name=f"psD{djc}_{tl}_{ncol}")
                                    djc, tl, ncol = self.cur
                                    toff = slice(tl * 128, (tl + 1) * 128)
                                    cc = slice(ncol * NC, (ncol + 1) * NC)
                                    take = min(n, H - self.pos)
                                    for h in range(self.pos, self.pos + take):
                                        nc.tensor.matmul(
                                            self.ps[:], self.ats[h][:, toff],
                                            wo_t[h][:, cc],
                                            start=(h == 0), stop=(h == H - 1))
                                    self.pos += take
                                    n -= take
                                    if self.pos == H:
                                        tt = slice(djc * NC + tl * 128,
                                                   djc * NC + (tl + 1) * 128)
                                        st = outst_pool.tile(
                                            [128, NC], BF, tag="outst",
                                            name=f"outst{djc}_{tl}_{ncol}")
                                        nc.scalar.copy(st[:], self.ps[:])
                                        nc.sync.dma_start(out=out_d[tt, cc],
                                                          in_=st[:])
                                        self.ps = None

                        dchain = DChain()
                        prev_ats = None
                        for jc in range(JC):
                            jj = slice(jc * NC, (jc + 1) * NC)
                            ats_t = []
                            pending = None
                            for h in range(H):
                                if prev_ats is not None:
                                    for k in range(4):
                                        idx = h * 4 + k
                                        dchain.add(jc - 1, idx // ND,
                                                   idx % ND, prev_ats)
                                ets = []
                                for i2 in range(SK // 2):
                                    ps2 = ps_pool.tile(
                                        [128, 2 * NC], F32, tag="sc",
                                        name=f"sc{h}_{jc}_{i2}")
                                    for p in range(2):
                                        i = i2 * 2 + p
                                        nc.tensor.matmul(
                                            ps2[:, p * NC:(p + 1) * NC],
                                            kT_t[h][:, i * 128:(i + 1) * 128],
                                            qT_t[h][:, jj],
                                            start=True, stop=True)
                                    # 4 out-proj matmuls of the PREVIOUS
                                    # query chunk between score pairs: ready
                                    # PE work that absorbs the exp drain.
                                    dchain.emit(4)
                                    et = et_pool.tile([128, 2 * NC], BF,
                                                      tag="ET",
                                                      name=f"et{h}_{jc}_{i2}")
                                    nc.scalar.activation(et[:], ps2[:], Exp)
                                    ets.append(et)
                                # previous head's norm ops go FIRST so its
                                # ats never queues behind this head's esum
                                # chain on DVE (pv buffer reuse waits on ats)
                                if pending is not None:
                                    emit_norm(jc, h - 1, *pending, ats_t)
                                # DVE tree-sum of the exp tiles for the
                                # softmax denominator.
                                es = esum_pool.tile([128, 2 * NC], BF,
                                                    tag="es", name=f"es{h}_{jc}")
                                nc.vector.tensor_add(es[:], ets[0][:], ets[1][:])
                                for i2 in range(2, SK // 2):
                                    nc.vector.tensor_add(es[:], es[:], ets[i2][:])
                                esf = esum_pool.tile([128, NC], BF, tag="esf",
                                                     name=f"esf{h}_{jc}")
                                nc.vector.tensor_add(esf[:], es[:, 0:NC],
                                                     es[:, NC:2 * NC])
                                pv = ps_pool.tile([Dh, NC], F32, tag="pv",
                                                    name=f"pv{h}_{jc}")
                                for i2 in range(SK // 2):
                                    for p in range(2):
                                        i = i2 * 2 + p
                                        sl = ets[i2][:, p * NC:(p + 1) * NC]
                                        nc.tensor.matmul(
                                            pv[:],
                                            v_t[i][:, h * Dh:(h + 1) * Dh],
                                            sl, start=(i == 0),
                                            stop=(i == SK - 1))
                                pending = (esf, pv)
                            emit_norm(jc, H - 1, *pending, ats_t)
                            prev_ats = ats_t

                        # out-projection for the last query chunk
                        for tl in range(NC // 128):
                            for ncol in range(ND):
                                dchain.add(JC - 1, tl, ncol, prev_ats)
                        dchain.emit(10 ** 9)
                    ps_cm.__exit__(None, None, None)

    nc.compile()
    return nc


def host_inputs(hidden_states, Wq, Wc, Wk, Wv, Wo, S=SEQ, Dh=HEAD_DIM,
                heads_per_core=HEADS_PER_CORE, n_cores=N_CORES):
    """Shard + preprocess full fp32 inputs into per-core bf16 in_maps."""
    scale = 1.0 / np.sqrt(Dh)
    pos = np.arange(S, dtype=np.float32)
    inv_freq = 1.0 / (ROPE_THETA ** (np.arange(0, Dh, 2, dtype=np.float32) / Dh))
    freqs = pos[:, None] * inv_freq
    emb = np.concatenate([freqs, freqs], axis=-1)      # [S, Dh]
    cosT = np.cos(emb).T.copy()                        # [Dh, S]
    sinT = np.sin(emb).T.copy()
    sinT[: Dh // 2] *= -1.0                            # sign baked for the swap trick
    cosq = (cosT * scale).astype(BF_NP)
    sinq = (sinT * scale).astype(BF_NP)
    cosk = cosT.astype(BF_NP)
    sink = sinT.astype(BF_NP)

    hw = heads_per_core * Dh
    in_maps = []
    for c in range(n_cores):
        b, g = divmod(c, 4)
        cols = slice(g * hw, (g + 1) * hw)
        in_maps.append({
            "hsT": np.ascontiguousarray(hidden_states[b].T).astype(BF_NP),
            "Wq": np.ascontiguousarray(Wq[:, cols]).astype(BF_NP),
            "Wc": Wc.astype(BF_NP),
            "Wk": np.ascontiguousarray(Wk[:, cols]).astype(BF_NP),
            "Wv": np.ascontiguousarray(Wv[:, cols]).astype(BF_NP),
            "Wo": np.ascontiguousarray(Wo[cols, :]).astype(BF_NP),
            "cosq": cosq, "sinq": sinq, "cosk": cosk, "sink": sink,
        })
    return in_maps


_NC_CACHE = {}


def kernel(hidden_states, Wq, Wc, Wk, Wv, Wo):
    hidden_states = np.asarray(hidden_states, dtype=np.float32)
    if "nc" not in _NC_CACHE:
        _NC_CACHE["nc"] = build_nc()
    nc = _NC_CACHE["nc"]
    in_maps = host_inputs(hidden_states, np.asarray(Wq, np.float32),
                          np.asarray(Wc, np.float32), np.asarray(Wk, np.float32),
                          np.asarray(Wv, np.float32), np.asarray(Wo, np.float32))
    res = run_bass_kernel_spmd(nc, in_maps, list(range(N_CORES))).results
    B, S, D = BATCH, SEQ, D_MODEL
    out = np.zeros((B, S, D), dtype=np.float32)
    for c in range(N_CORES):
        out[c // 4] += res[c]["out"]
    return out
